# revision 19
# baseline (speedup 1.0000x reference)
"""Additive attention (B=4, Q=KV=512, H=256) on 8 Trainium2 NeuronCores.

Math (per batch b):
  q = queries @ W_q            (Q, H)
  k = keys    @ W_k            (KV, H)
  scores[i,j] = sum_h w_v[h] * tanh(q[i,h] + k[j,h])
  attn = softmax_j(scores masked to j < valid_lens[b])
  out  = attn @ values         (Q, V)

Sharding: every core takes query rows [c*64, (c+1)*64) of EVERY batch.
That keeps all 8 cores perfectly balanced and the SPMD program uniform even
though the per-batch key window (truncated to ceil(valid/32)*32 columns --
masked columns contribute exactly 0 after softmax) differs per batch.

Device layout: h on partitions for the tanh stage.  For each query row i,
S[h, j] = k[h, j] + q[h, i] is one DVE tensor_scalar_add (per-partition
scalar broadcast); tanh runs in-place on ScalarE over row-blocks.  The
w_v-weighted reduction over h produces scores TRANSPOSED -- for each
(row, 128-wide j-chunk, h-half) one TensorE matmul with the tanh tile as
stationary and the w_v column as the moving operand writes scores_T[j, i]
into PSUM (partition base 0, always legal).  Softmax then works in the
transposed layout: exp(x + mask) is a single ScalarE activation with the
additive mask as per-partition bias, row sums come from a ones-vector
matmul, and the unnormalized exp_T feeds the final values matmul directly
as lhsT (no attention transpose at all); the 1/sum scale is applied to the
output rows as a per-partition DVE scale.
"""

import sys
import types

import numpy as np

NEG = -1.0e6
NCORES = 8
TRACE = False  # test.py flips this to get a profiled run
LAST_RESULT = None  # BassKernelResults stash for test.py


def _install_axon_profile_hook():
    """antenv.axon_hooks is missing from this image; concourse needs it for
    trace=True under axon. Register the ctypes-based NTFF hook manually."""
    import antenv

    if "antenv.axon_hooks" in sys.modules:
        return
    m = types.ModuleType("antenv.axon_hooks")
    m._hook = None

    def _set(h):
        m._hook = h

    def _get():
        return m._hook

    m.set_axon_ntff_profile_hook = _set
    m.get_axon_ntff_profile_hook = _get
    sys.modules["antenv.axon_hooks"] = m
    antenv.axon_hooks = m
    try:
        from trn_agent_boot.trn_boot import _ntff_profile_via_ctypes

        m.set_axon_ntff_profile_hook(
            _ntff_profile_via_ctypes("/opt/axon/libaxon_pjrt.so")
        )
    except Exception:
        pass


def _patch_tile_drain():
    """The walrus build in this image allows at most ONE sync-wait command
    per instruction; Tile's kernel-tail drain carries every vector-clock
    wait on a single drain. Split them across a chain of drains."""
    import concourse.mybir as mybir
    import concourse.tile as tile
    from concourse.vector_clock import ScopedClock

    if getattr(tile.TileContext, "_drain_patched", False):
        return

    def _drain_and_barrier_chunked(self, tick_clock, wait_clock):
        d0 = self.nc.sync.drain()
        wait_clock.add_sem_waits(d0.ins, ScopedClock({None: tick_clock.global_clock}))
        si = d0.ins.sync_info
        waits = list(si.on_wait) if si is not None else []
        if len(waits) > 1:
            # spread the waits round-robin over all engine streams (each
            # instruction may carry at most one wait for this walrus; a
            # serial SP chain would cost ~27 x wait-resolve latency). The
            # all_engine_barrier right after makes the join equivalent.
            engs = [
                mybir.EngineType.SP,
                mybir.EngineType.DVE,
                mybir.EngineType.Activation,
                mybir.EngineType.PE,
                mybir.EngineType.Pool,
            ]
            d0.ins.sync_info = mybir.SyncInfo(
                on_wait=waits[:1], on_update=list(si.on_update)
            )
            for i in range(1, len(waits)):
                ev = mybir.InstEventSemaphore(
                    name=f"tail-wait-{i}",
                    engine=engs[i % len(engs)],
                    ins=[],
                    outs=[],
                    sync_info=mybir.SyncInfo(on_wait=[waits[i]], on_update=[]),
                )
                self.nc.register_instruction(ev)
                self.nc.cur_bb.bb.add_instruction(ev)

        self.nc.all_engine_barrier()
        assert self.sems is not None
        popped = self.nc._tile_sem_poison_stack.pop()
        assert popped is self._sem_poison
        self.nc.clear_and_free_semaphores(list(self.sems.allocated().values()))
        self.nc.all_engine_barrier()

    tile.TileContext._drain_and_barrier = _drain_and_barrier_chunked
    tile.TileContext._drain_patched = True


def _split_multi_waits(nc):
    """walrus here allows one sync-wait command per instruction; move extra
    waits onto standalone EventSemaphore instructions (same engine, just
    before the original instruction -- semantically identical since waits
    are monotonic sem-ge conditions)."""
    import concourse.mybir as mybir

    n = 0
    for fn in nc.m.functions:
        for blk in fn.blocks:
            out = []
            for inst in blk.instructions:
                si = inst.sync_info
                waits = list(si.on_wait) if si is not None else []
                if len(waits) > 1:
                    for k, w in enumerate(waits[:-1]):
                        ev = mybir.InstEventSemaphore(
                            name=f"{inst.name}-xw{k}",
                            engine=inst.engine,
                            ins=[],
                            outs=[],
                            sync_info=mybir.SyncInfo(on_wait=[w], on_update=[]),
                        )
                        out.append(ev)
                        n += 1
                    inst.sync_info = mybir.SyncInfo(
                        on_wait=[waits[-1]], on_update=list(si.on_update)
                    )
                out.append(inst)
            blk.instructions = out
    return n


def _ceil_to(x, m):
    return -(-int(x) // m) * m


def _row_block(IB, jmax, last=False):
    """Rows per tanh block (divisor of IB). Wide batches get 32-row blocks
    (fewer ACT per-call overheads); the last-processed batch stays at 16 so
    its trailing score matmuls don't stretch the kernel tail."""
    cap = 16 if (last or jmax < 128) else 32
    rb = max(1, min(IB, cap, 12288 // jmax))
    return 1 << (rb.bit_length() - 1)


def _build_program(B, Q, D, KV, V, H, jmaxs, IB):
    """One Bass program, shared by all 8 cores (SPMD; data differs per core).

    jmaxs[b]: truncated key-window width for batch b (multiple of 32).
    IB: query rows per (core, batch) = Q // NCORES.
    """
    import contextlib

    import concourse.bass as bass
    import concourse.mybir as mybir
    import concourse.tile as tile

    f32 = mybir.dt.float32
    bf16 = mybir.dt.bfloat16
    AF = mybir.ActivationFunctionType

    JTOT = int(np.sum(jmaxs))
    joff = np.concatenate([[0], np.cumsum(jmaxs)]).astype(int)  # key-col offsets
    # values are packed per batch at 128-row boundaries (slot layout)
    jpads = [_ceil_to(j, 128) for j in jmaxs]
    voff = np.concatenate([[0], np.cumsum(jpads)]).astype(int)
    VTOT = int(voff[-1])
    nchs = [_ceil_to(j, 128) // 128 for j in jmaxs]  # j-chunks per batch
    moff = np.concatenate([[0], np.cumsum(nchs)]).astype(int)  # maskT col offsets
    NCHTOT = int(moff[-1])
    NQROWS = B * IB  # query rows per core
    DC = D // 128  # contraction chunks for the projections
    HC = H // 128  # h-halves

    # processing order: widest batch first, narrowest last -- the epilogues
    # are software-pipelined one batch behind, so the tail is the (short)
    # last batch's epilogue chain.
    order = list(np.argsort(jmaxs))[::-1]

    nc = bass.Bass("TRN2", target_bir_lowering=False)
    d_queriesT = nc.declare_dram_parameter("queriesT", [D, NQROWS], bf16, isOutput=False)
    d_keysT = nc.declare_dram_parameter("keysT", [D, JTOT], bf16, isOutput=False)
    d_values = nc.declare_dram_parameter("values_p", [VTOT, V], bf16, isOutput=False)
    d_wq = nc.declare_dram_parameter("W_q", [D, H], bf16, isOutput=False)
    d_wk = nc.declare_dram_parameter("W_k", [D, H], bf16, isOutput=False)
    d_wv = nc.declare_dram_parameter("wv2", [128, HC], f32, isOutput=False)
    d_maskT = nc.declare_dram_parameter("maskT", [128, NCHTOT], f32, isOutput=False)
    d_out = nc.declare_dram_parameter("out", [NQROWS, V], f32, isOutput=True)

    with tile.TileContext(nc) as tc:
        ctx = contextlib.ExitStack()
        with ctx:
            const_pool = ctx.enter_context(tc.tile_pool(name="const", bufs=1))
            w_pool = ctx.enter_context(tc.tile_pool(name="w", bufs=1))
            in_pool = ctx.enter_context(tc.tile_pool(name="in", bufs=1))
            proj_pool = ctx.enter_context(tc.tile_pool(name="proj", bufs=1))

            wv_sb = const_pool.tile([128, HC], f32)
            nc.gpsimd.dma_start(out=wv_sb[:], in_=d_wv[:])
            wv_bf = const_pool.tile([128, HC], bf16)
            nc.vector.tensor_copy(wv_bf[:], wv_sb[:])
            maskT_sb = const_pool.tile([128, NCHTOT], f32)
            nc.gpsimd.dma_start(out=maskT_sb[:], in_=d_maskT[:])
            ones_sb = const_pool.tile([128, 1], f32)
            nc.gpsimd.memset(ones_sb[:], 1.0)
            ones_bf = const_pool.tile([128, 1], bf16)
            nc.gpsimd.memset(ones_bf[:], 1.0)
            warm = const_pool.tile([1, 1], f32)
            nc.scalar.activation(warm[0:1, 0:1], ones_sb[0:1, 0:1], AF.Tanh)

            kT_all = in_pool.tile([128, DC * JTOT], bf16, tag="kT", name="kT_all")
            wq_all = w_pool.tile([128, DC * H], bf16, tag="wq", name="wq_all")
            wk_all = w_pool.tile([128, DC * H], bf16, tag="wk", name="wk_all")
            qT_all = in_pool.tile([128, DC * NQROWS], bf16, tag="qT", name="qT_all")

            def kt_3d(jo, jm):
                base = kT_all[:]
                return bass.AP(
                    base.tensor, base.offset + jo, [base.ap[0], [JTOT, DC], [1, jm]]
                )

            # one wide DMA per logical tensor: per-tensor 3D access patterns
            # put the dc-chunks side by side in SBUF; the serial ~600ns
            # per-dma_start sequencer issue cost was dominating the head.
            b0p = order[0]
            nc.sync.dma_start(
                out=kt_3d(int(joff[b0p]), int(jmaxs[b0p])),
                in_=d_keysT[:, joff[b0p]:joff[b0p] + jmaxs[b0p]].rearrange(
                    "(dc p) j -> p dc j", p=128
                ),
            )
            nc.sync.dma_start(
                out=wk_all[:].rearrange("p (dc h) -> p dc h", h=H),
                in_=d_wk.rearrange("(dc p) h -> p dc h", p=128),
            )
            nc.scalar.dma_start(
                out=wq_all[:].rearrange("p (dc h) -> p dc h", h=H),
                in_=d_wq.rearrange("(dc p) h -> p dc h", p=128),
            )
            nc.scalar.dma_start(
                out=qT_all[:].rearrange("p (dc r) -> p dc r", r=NQROWS),
                in_=d_queriesT.rearrange("(dc p) r -> p dc r", p=128),
            )
            for b in order[1:]:
                nc.sync.dma_start(
                    out=kt_3d(int(joff[b]), int(jmaxs[b])),
                    in_=d_keysT[:, joff[b]:joff[b] + jmaxs[b]].rearrange(
                        "(dc p) j -> p dc j", p=128
                    ),
                )

            values_sb = in_pool.tile([128, (VTOT // 128) * V], bf16, tag="vals")
            nc.gpsimd.dma_start(
                out=values_sb[:].rearrange("p (s v) -> p s v", v=V),
                in_=d_values.rearrange("(s p) v -> p s v", p=128),
            )

            # ---- projections (per batch window, first-processed first)
            # q/k slabs feed the DVE broadcast-add: k in bf16 (4x DVE mode),
            # q stays f32 (tensor_scalar scalar operand must be f32)
            q_sb = [
                proj_pool.tile([128, NQROWS], f32, tag=f"q{hc}", name=f"qsb{hc}")
                for hc in range(HC)
            ]
            k_sb = [
                proj_pool.tile([128, JTOT], bf16, tag=f"k{hc}", name=f"ksb{hc}")
                for hc in range(HC)
            ]
            with tc.tile_pool(name="ppsum", bufs=2, space="PSUM") as ppsum:
                def proj_k(b, hcs=None):
                    jo, jm = int(joff[b]), int(jmaxs[b])
                    for hc in hcs if hcs is not None else range(HC):
                        pk = ppsum.tile([128, 512], f32, tag="pproj", name="pk")
                        for dc in range(DC):
                            nc.tensor.matmul(
                                pk[:, :jm],
                                wk_all[:, dc * H + hc * 128:dc * H + (hc + 1) * 128],
                                kT_all[:, dc * JTOT + jo:dc * JTOT + jo + jm],
                                start=(dc == 0),
                                stop=(dc == DC - 1),
                            )
                        nc.vector.tensor_copy(k_sb[hc][:, jo:jo + jm], pk[:, :jm])

                def proj_q(hc):
                    pq = ppsum.tile([128, NQROWS], f32, tag="pproj", name="pq")
                    for dc in range(DC):
                        nc.tensor.matmul(
                            pq[:],
                            wq_all[:, dc * H + hc * 128:dc * H + (hc + 1) * 128],
                            qT_all[:, dc * NQROWS:(dc + 1) * NQROWS],
                            start=(dc == 0),
                            stop=(dc == DC - 1),
                        )
                    nc.vector.tensor_copy(q_sb[hc][:], pq[:])

                proj_k(order[0], hcs=[0])
                proj_q(0)
                proj_k(order[0], hcs=list(range(1, HC)))
                for hc in range(1, HC):
                    proj_q(hc)
                for b in order[1:]:
                    proj_k(b)

            # ---- main: tanh features -> transposed scores -> softmax -> out
            s_pool = ctx.enter_context(tc.tile_pool(name="S", bufs=4))
            sc_psum = ctx.enter_context(tc.tile_pool(name="scps", bufs=5, space="PSUM"))
            sm_psum = ctx.enter_context(tc.tile_pool(name="smps", bufs=2, space="PSUM"))
            o_psum = ctx.enter_context(tc.tile_pool(name="ops", bufs=1, space="PSUM"))
            soft_pool = ctx.enter_context(tc.tile_pool(name="soft", bufs=4))
            out_pool = ctx.enter_context(tc.tile_pool(name="outp", bufs=2))

            def epilogue(b, psc):
                jmax = int(jmaxs[b])
                nch = nchs[b]
                lns = [min(128, jmax - jc * 128) for jc in range(nch)]
                eT = [
                    soft_pool.tile([128, IB], bf16, tag="eT", name=f"eT{b}_{jc}")
                    for jc in range(nch)
                ]
                for jc in range(nch):
                    nc.scalar.activation(
                        eT[jc][: lns[jc], :],
                        psc[jc][: lns[jc], :],
                        AF.Exp,
                        bias=maskT_sb[: lns[jc], moff[b] + jc:moff[b] + jc + 1],
                    )
                psums = sm_psum.tile([1, IB], f32, tag="sm", name=f"psums{b}")
                for jc in range(nch):
                    nc.tensor.matmul(
                        psums[0:1, :],
                        ones_bf[: lns[jc], 0:1],
                        eT[jc][: lns[jc], :],
                        start=(jc == 0),
                        stop=(jc == nch - 1),
                    )
                rs = soft_pool.tile([1, IB], f32, tag="rs", name=f"rs{b}")
                nc.vector.reciprocal(rs[0:1, :], psums[0:1, :])
                prt = sm_psum.tile([IB, 1], f32, tag="sm", name=f"prt{b}")
                nc.tensor.matmul(
                    prt[:, 0:1], rs[0:1, :], ones_sb[0:1, 0:1], start=True, stop=True
                )
                rinv = soft_pool.tile([IB, 1], f32, tag="rinv", name=f"rinv{b}")
                nc.vector.tensor_copy(rinv[:], prt[:])

                pout = o_psum.tile([IB, V], f32, tag="pout", name=f"pout{b}")
                for jc in range(nch):
                    nc.tensor.matmul(
                        pout[:],
                        eT[jc][: lns[jc], :],
                        values_sb[: lns[jc], (voff[b] // 128 + jc) * V:(voff[b] // 128 + jc + 1) * V],
                        start=(jc == 0),
                        stop=(jc == nch - 1),
                    )
                out_sb = out_pool.tile([IB, V], f32, tag="osb", name=f"osb{b}")
                nc.vector.tensor_scalar_mul(out_sb[:], pout[:], rinv[:])
                nc.sync.dma_start(out=d_out[b * IB:(b + 1) * IB, :], in_=out_sb[:])

            pending = None  # (b, psc) whose epilogue is deferred one batch
            for b in order:
                jmax = int(jmaxs[b])
                jo = int(joff[b])
                nch = nchs[b]
                lns = [min(128, jmax - jc * 128) for jc in range(nch)]
                RB = _row_block(IB, jmax, last=(b == order[-1]))

                psc = [
                    sc_psum.tile([128, IB], f32, tag="pscT", name=f"pscT{b}_{jc}")
                    for jc in range(nch)
                ]
                for r0 in range(0, IB, RB):
                    S = [
                        s_pool.tile(
                            [128, RB * jmax], bf16, tag="S", name=f"S{b}_{r0}_{hcx}"
                        )
                        for hcx in range(HC)
                    ]
                    F = [
                        s_pool.tile(
                            [128, RB * jmax], bf16, tag="F", name=f"F{b}_{r0}_{hcx}"
                        )
                        for hcx in range(HC)
                    ]
                    for hc in range(HC):
                        if jmax <= 128:
                            # narrow window: one broadcast tensor-tensor add
                            # covers the whole row block (per-call DVE
                            # overhead would dominate row-by-row adds)
                            kb = k_sb[hc][:, jo:jo + jmax]
                            k_rep = bass.AP(
                                kb.tensor, kb.offset, [kb.ap[0], [0, RB], kb.ap[1]]
                            )
                            qb = q_sb[hc][:, b * IB + r0:b * IB + r0 + RB]
                            q_rep = bass.AP(
                                qb.tensor, qb.offset, [qb.ap[0], qb.ap[1], [0, jmax]]
                            )
                            sb = S[hc][:, : RB * jmax]
                            s3 = bass.AP(
                                sb.tensor, sb.offset, [sb.ap[0], [jmax, RB], [1, jmax]]
                            )
                            nc.vector.tensor_add(s3, k_rep, q_rep)
                        else:
                            for m in range(RB):
                                row = b * IB + r0 + m
                                nc.vector.tensor_scalar_add(
                                    S[hc][:, m * jmax:(m + 1) * jmax],
                                    k_sb[hc][:, jo:jo + jmax],
                                    q_sb[hc][:, row:row + 1],
                                )
                        # bf16 tanh output: full-128-col weights trigger the
                        # compiler-automatic FWL fast-weight-load path
                        nc.scalar.activation(F[hc][:], S[hc][:], AF.Tanh)
                    for m in range(RB):
                        for jc in range(nch):
                            for hc in range(HC):
                                nc.tensor.matmul(
                                    psc[jc][: lns[jc], r0 + m:r0 + m + 1],
                                    F[hc][:, m * jmax + jc * 128:m * jmax + jc * 128 + lns[jc]],
                                    wv_bf[:, hc:hc + 1],
                                    start=(hc == 0),
                                    stop=(hc == HC - 1),
                                )
                    if r0 == 0 and pending is not None:
                        epilogue(*pending)
                        pending = None
                pending = (b, psc)
            epilogue(*pending)

    _split_multi_waits(nc)
    return nc


def kernel(queries, keys, values, valid_lens, W_q, W_k, w_v):
    global LAST_RESULT
    _install_axon_profile_hook()
    _patch_tile_drain()
    from concourse.bass_utils import run_bass_kernel_spmd

    import ml_dtypes

    bf = ml_dtypes.bfloat16
    queries = np.ascontiguousarray(queries, dtype=np.float32)
    keys = np.ascontiguousarray(keys, dtype=np.float32)
    values = np.ascontiguousarray(values, dtype=np.float32)
    W_q = np.ascontiguousarray(W_q, dtype=np.float32)
    W_k = np.ascontiguousarray(W_k, dtype=np.float32)
    w_v = np.ascontiguousarray(w_v, dtype=np.float32)
    vl = np.asarray(valid_lens).astype(np.int64)

    B, Q, D = queries.shape
    KV = keys.shape[1]
    V = values.shape[2]
    H = W_q.shape[1]
    IB = Q // NCORES
    HC = H // 128

    jmaxs = [min(KV, _ceil_to(max(int(v), 1), 32)) for v in vl]
    jpads = [_ceil_to(j, 128) for j in jmaxs]
    nchs = [j // 128 for j in jpads]
    VTOT = int(np.sum(jpads))

    nc = _build_program(B, Q, D, KV, V, H, jmaxs, IB)

    # ---- shared (core-independent) arrays
    keysT = np.concatenate(
        [keys[b, : jmaxs[b], :].T for b in range(B)], axis=1
    ).astype(bf)  # (D, JTOT)
    values_p = np.zeros((VTOT, V), bf)
    off = 0
    for b in range(B):
        values_p[off:off + jmaxs[b]] = values[b, : jmaxs[b], :].astype(bf)
        off += jpads[b]
    wv2 = w_v.reshape(HC, 128).T.copy()  # (128, HC)
    # additive mask in the transposed layout: one 128-long column per
    # (batch, j-chunk); row p of column (b, jc) corresponds to key j = jc*128+p
    mcols = []
    for b in range(B):
        for jc in range(nchs[b]):
            j = jc * 128 + np.arange(128)
            mcols.append(np.where(j < int(vl[b]), 0.0, NEG).astype(np.float32))
    maskT = np.stack(mcols, axis=1)  # (128, NCHTOT)

    in_maps = []
    for c in range(NCORES):
        queriesT = np.concatenate(
            [queries[b, c * IB:(c + 1) * IB, :].T for b in range(B)], axis=1
        )  # (D, B*IB)
        in_maps.append(
            {
                "queriesT": np.ascontiguousarray(queriesT.astype(bf)),
                "keysT": np.ascontiguousarray(keysT),
                "values_p": values_p,
                "W_q": W_q.astype(bf),
                "W_k": W_k.astype(bf),
                "wv2": wv2,
                "maskT": maskT,
            }
        )

    res = run_bass_kernel_spmd(
        nc, in_maps, core_ids=list(range(NCORES)), trace=TRACE
    )
    LAST_RESULT = res

    out = np.empty((B, Q, V), np.float32)
    for c in range(NCORES):
        o = res.results[c]["out"]  # (B*IB, V)
        for b in range(B):
            out[b, c * IB:(c + 1) * IB, :] = o[b * IB:(b + 1) * IB, :]
    return out


# revision 20
# speedup vs baseline: 1.0388x; 1.0388x over previous
"""Additive attention (B=4, Q=KV=512, H=256) on 8 Trainium2 NeuronCores.

Math (per batch b):
  q = queries @ W_q            (Q, H)
  k = keys    @ W_k            (KV, H)
  scores[i,j] = sum_h w_v[h] * tanh(q[i,h] + k[j,h])
  attn = softmax_j(scores masked to j < valid_lens[b])
  out  = attn @ values         (Q, V)

Sharding: every core takes query rows [c*64, (c+1)*64) of EVERY batch.
That keeps all 8 cores perfectly balanced and the SPMD program uniform even
though the per-batch key window (truncated to ceil(valid/32)*32 columns --
masked columns contribute exactly 0 after softmax) differs per batch.

Device layout: h on partitions for the tanh stage.  For each query row i,
S[h, j] = k[h, j] + q[h, i] is one DVE tensor_scalar_add (per-partition
scalar broadcast); tanh runs in-place on ScalarE over row-blocks.  The
w_v-weighted reduction over h produces scores TRANSPOSED -- for each
(row, 128-wide j-chunk, h-half) one TensorE matmul with the tanh tile as
stationary and the w_v column as the moving operand writes scores_T[j, i]
into PSUM (partition base 0, always legal).  Softmax then works in the
transposed layout: exp(x + mask) is a single ScalarE activation with the
additive mask as per-partition bias, row sums come from a ones-vector
matmul, and the unnormalized exp_T feeds the final values matmul directly
as lhsT (no attention transpose at all); the 1/sum scale is applied to the
output rows as a per-partition DVE scale.
"""

import sys
import types

import numpy as np

NEG = -1.0e6
NCORES = 8
TRACE = False  # test.py flips this to get a profiled run
LAST_RESULT = None  # BassKernelResults stash for test.py


def _install_axon_profile_hook():
    """antenv.axon_hooks is missing from this image; concourse needs it for
    trace=True under axon. Register the ctypes-based NTFF hook manually."""
    import antenv

    if "antenv.axon_hooks" in sys.modules:
        return
    m = types.ModuleType("antenv.axon_hooks")
    m._hook = None

    def _set(h):
        m._hook = h

    def _get():
        return m._hook

    m.set_axon_ntff_profile_hook = _set
    m.get_axon_ntff_profile_hook = _get
    sys.modules["antenv.axon_hooks"] = m
    antenv.axon_hooks = m
    try:
        from trn_agent_boot.trn_boot import _ntff_profile_via_ctypes

        m.set_axon_ntff_profile_hook(
            _ntff_profile_via_ctypes("/opt/axon/libaxon_pjrt.so")
        )
    except Exception:
        pass


def _patch_tile_drain():
    """The walrus build in this image allows at most ONE sync-wait command
    per instruction; Tile's kernel-tail drain carries every vector-clock
    wait on a single drain. Split them across a chain of drains."""
    import concourse.mybir as mybir
    import concourse.tile as tile
    from concourse.vector_clock import ScopedClock

    if getattr(tile.TileContext, "_drain_patched", False):
        return

    def _drain_and_barrier_chunked(self, tick_clock, wait_clock):
        d0 = self.nc.sync.drain()
        wait_clock.add_sem_waits(d0.ins, ScopedClock({None: tick_clock.global_clock}))
        si = d0.ins.sync_info
        waits = list(si.on_wait) if si is not None else []
        if len(waits) > 1:
            # spread the waits round-robin over all engine streams (each
            # instruction may carry at most one wait for this walrus; a
            # serial SP chain would cost ~27 x wait-resolve latency). The
            # all_engine_barrier right after makes the join equivalent.
            engs = [
                mybir.EngineType.SP,
                mybir.EngineType.DVE,
                mybir.EngineType.Activation,
                mybir.EngineType.PE,
                mybir.EngineType.Pool,
            ]
            d0.ins.sync_info = mybir.SyncInfo(
                on_wait=waits[:1], on_update=list(si.on_update)
            )
            for i in range(1, len(waits)):
                ev = mybir.InstEventSemaphore(
                    name=f"tail-wait-{i}",
                    engine=engs[i % len(engs)],
                    ins=[],
                    outs=[],
                    sync_info=mybir.SyncInfo(on_wait=[waits[i]], on_update=[]),
                )
                self.nc.register_instruction(ev)
                self.nc.cur_bb.bb.add_instruction(ev)

        self.nc.all_engine_barrier()
        assert self.sems is not None
        popped = self.nc._tile_sem_poison_stack.pop()
        assert popped is self._sem_poison
        self.nc.clear_and_free_semaphores(list(self.sems.allocated().values()))
        self.nc.all_engine_barrier()

    tile.TileContext._drain_and_barrier = _drain_and_barrier_chunked
    tile.TileContext._drain_patched = True


def _split_multi_waits(nc):
    """walrus here allows one sync-wait command per instruction; move extra
    waits onto standalone EventSemaphore instructions (same engine, just
    before the original instruction -- semantically identical since waits
    are monotonic sem-ge conditions)."""
    import concourse.mybir as mybir

    n = 0
    for fn in nc.m.functions:
        for blk in fn.blocks:
            out = []
            for inst in blk.instructions:
                si = inst.sync_info
                waits = list(si.on_wait) if si is not None else []
                if len(waits) > 1:
                    for k, w in enumerate(waits[:-1]):
                        ev = mybir.InstEventSemaphore(
                            name=f"{inst.name}-xw{k}",
                            engine=inst.engine,
                            ins=[],
                            outs=[],
                            sync_info=mybir.SyncInfo(on_wait=[w], on_update=[]),
                        )
                        out.append(ev)
                        n += 1
                    inst.sync_info = mybir.SyncInfo(
                        on_wait=[waits[-1]], on_update=list(si.on_update)
                    )
                out.append(inst)
            blk.instructions = out
    return n


def _ceil_to(x, m):
    return -(-int(x) // m) * m


def _row_block(IB, jmax, last=False):
    """Rows per tanh block: keep ACT calls ~2-6K elems/lane (divisor of IB).
    Capped at 16 rows so the DVE->ACT->PE pipeline stays fine-grained."""
    rb = max(1, min(IB, 16, 6144 // jmax))
    return 1 << (rb.bit_length() - 1)


def _build_program(B, Q, D, KV, V, H, jmaxs, IB):
    """One Bass program, shared by all 8 cores (SPMD; data differs per core).

    jmaxs[b]: truncated key-window width for batch b (multiple of 32).
    IB: query rows per (core, batch) = Q // NCORES.
    """
    import contextlib

    import concourse.bass as bass
    import concourse.mybir as mybir
    import concourse.tile as tile

    f32 = mybir.dt.float32
    bf16 = mybir.dt.bfloat16
    AF = mybir.ActivationFunctionType

    JTOT = int(np.sum(jmaxs))
    joff = np.concatenate([[0], np.cumsum(jmaxs)]).astype(int)  # key-col offsets
    # values are packed per batch at 128-row boundaries (slot layout)
    jpads = [_ceil_to(j, 128) for j in jmaxs]
    voff = np.concatenate([[0], np.cumsum(jpads)]).astype(int)
    VTOT = int(voff[-1])
    nchs = [_ceil_to(j, 128) // 128 for j in jmaxs]  # j-chunks per batch
    moff = np.concatenate([[0], np.cumsum(nchs)]).astype(int)  # maskT col offsets
    NCHTOT = int(moff[-1])
    NQROWS = B * IB  # query rows per core
    DC = D // 128  # contraction chunks for the projections
    HC = H // 128  # h-halves

    # processing order: widest batch first, narrowest last -- the epilogues
    # are software-pipelined one batch behind, so the tail is the (short)
    # last batch's epilogue chain.
    order = list(np.argsort(jmaxs))[::-1]

    nc = bass.Bass("TRN2", target_bir_lowering=False)
    d_queriesT = nc.declare_dram_parameter("queriesT", [D, NQROWS], bf16, isOutput=False)
    d_keysT = nc.declare_dram_parameter("keysT", [D, JTOT], bf16, isOutput=False)
    d_values = nc.declare_dram_parameter("values_p", [VTOT, V], bf16, isOutput=False)
    d_wq = nc.declare_dram_parameter("W_q", [D, H], bf16, isOutput=False)
    d_wk = nc.declare_dram_parameter("W_k", [D, H], bf16, isOutput=False)
    d_wv = nc.declare_dram_parameter("wv2", [128, HC], f32, isOutput=False)
    d_maskT = nc.declare_dram_parameter("maskT", [128, NCHTOT], f32, isOutput=False)
    d_out = nc.declare_dram_parameter("out", [NQROWS, V], f32, isOutput=True)

    with tile.TileContext(nc) as tc:
        ctx = contextlib.ExitStack()
        with ctx:
            const_pool = ctx.enter_context(tc.tile_pool(name="const", bufs=1))
            w_pool = ctx.enter_context(tc.tile_pool(name="w", bufs=1))
            in_pool = ctx.enter_context(tc.tile_pool(name="in", bufs=1))
            proj_pool = ctx.enter_context(tc.tile_pool(name="proj", bufs=1))

            wv_sb = const_pool.tile([128, HC], f32)
            nc.gpsimd.dma_start(out=wv_sb[:], in_=d_wv[:])
            wv_bf = const_pool.tile([128, HC], bf16)
            nc.vector.tensor_copy(wv_bf[:], wv_sb[:])
            maskT_sb = const_pool.tile([128, NCHTOT], f32)
            nc.gpsimd.dma_start(out=maskT_sb[:], in_=d_maskT[:])
            ones_sb = const_pool.tile([128, 1], f32)
            nc.gpsimd.memset(ones_sb[:], 1.0)
            ones_bf = const_pool.tile([128, 1], bf16)
            nc.gpsimd.memset(ones_bf[:], 1.0)
            warm = const_pool.tile([1, 1], f32)
            nc.scalar.activation(warm[0:1, 0:1], ones_sb[0:1, 0:1], AF.Tanh)

            kT_all = in_pool.tile([128, DC * JTOT], bf16, tag="kT", name="kT_all")
            wq_all = w_pool.tile([128, DC * H], bf16, tag="wq", name="wq_all")
            wk_all = w_pool.tile([128, DC * H], bf16, tag="wk", name="wk_all")
            qT_all = in_pool.tile([128, DC * NQROWS], bf16, tag="qT", name="qT_all")

            def kt_3d(jo, jm):
                base = kT_all[:]
                return bass.AP(
                    base.tensor, base.offset + jo, [base.ap[0], [JTOT, DC], [1, jm]]
                )

            # one wide DMA per logical tensor: per-tensor 3D access patterns
            # put the dc-chunks side by side in SBUF; the serial ~600ns
            # per-dma_start sequencer issue cost was dominating the head.
            b0p = order[0]
            nc.sync.dma_start(
                out=kt_3d(int(joff[b0p]), int(jmaxs[b0p])),
                in_=d_keysT[:, joff[b0p]:joff[b0p] + jmaxs[b0p]].rearrange(
                    "(dc p) j -> p dc j", p=128
                ),
            )
            nc.sync.dma_start(
                out=wk_all[:].rearrange("p (dc h) -> p dc h", h=H),
                in_=d_wk.rearrange("(dc p) h -> p dc h", p=128),
            )
            nc.scalar.dma_start(
                out=wq_all[:].rearrange("p (dc h) -> p dc h", h=H),
                in_=d_wq.rearrange("(dc p) h -> p dc h", p=128),
            )
            nc.scalar.dma_start(
                out=qT_all[:].rearrange("p (dc r) -> p dc r", r=NQROWS),
                in_=d_queriesT.rearrange("(dc p) r -> p dc r", p=128),
            )
            for b in order[1:]:
                nc.sync.dma_start(
                    out=kt_3d(int(joff[b]), int(jmaxs[b])),
                    in_=d_keysT[:, joff[b]:joff[b] + jmaxs[b]].rearrange(
                        "(dc p) j -> p dc j", p=128
                    ),
                )

            values_sb = in_pool.tile([128, (VTOT // 128) * V], bf16, tag="vals")
            nc.gpsimd.dma_start(
                out=values_sb[:].rearrange("p (s v) -> p s v", v=V),
                in_=d_values.rearrange("(s p) v -> p s v", p=128),
            )

            # ---- projections (per batch window, first-processed first)
            # q/k slabs feed the DVE broadcast-add: k in bf16 (4x DVE mode),
            # q stays f32 (tensor_scalar scalar operand must be f32)
            q_sb = [
                proj_pool.tile([128, NQROWS], f32, tag=f"q{hc}", name=f"qsb{hc}")
                for hc in range(HC)
            ]
            k_sb = [
                proj_pool.tile([128, JTOT], bf16, tag=f"k{hc}", name=f"ksb{hc}")
                for hc in range(HC)
            ]
            with tc.tile_pool(name="ppsum", bufs=2, space="PSUM") as ppsum:
                def proj_k(b, hcs=None):
                    jo, jm = int(joff[b]), int(jmaxs[b])
                    for hc in hcs if hcs is not None else range(HC):
                        pk = ppsum.tile([128, 512], f32, tag="pproj", name="pk")
                        for dc in range(DC):
                            nc.tensor.matmul(
                                pk[:, :jm],
                                wk_all[:, dc * H + hc * 128:dc * H + (hc + 1) * 128],
                                kT_all[:, dc * JTOT + jo:dc * JTOT + jo + jm],
                                start=(dc == 0),
                                stop=(dc == DC - 1),
                            )
                        nc.vector.tensor_copy(k_sb[hc][:, jo:jo + jm], pk[:, :jm])

                def proj_q(hc):
                    pq = ppsum.tile([128, NQROWS], f32, tag="pproj", name="pq")
                    for dc in range(DC):
                        nc.tensor.matmul(
                            pq[:],
                            wq_all[:, dc * H + hc * 128:dc * H + (hc + 1) * 128],
                            qT_all[:, dc * NQROWS:(dc + 1) * NQROWS],
                            start=(dc == 0),
                            stop=(dc == DC - 1),
                        )
                    nc.vector.tensor_copy(q_sb[hc][:], pq[:])

                proj_k(order[0], hcs=[0])
                proj_q(0)
                proj_k(order[0], hcs=list(range(1, HC)))
                for hc in range(1, HC):
                    proj_q(hc)
                for b in order[1:]:
                    proj_k(b)

            # ---- main: tanh features -> transposed scores -> softmax -> out
            s_pool = ctx.enter_context(tc.tile_pool(name="S", bufs=8))
            sc_psum = ctx.enter_context(tc.tile_pool(name="scps", bufs=5, space="PSUM"))
            sm_psum = ctx.enter_context(tc.tile_pool(name="smps", bufs=2, space="PSUM"))
            o_psum = ctx.enter_context(tc.tile_pool(name="ops", bufs=1, space="PSUM"))
            soft_pool = ctx.enter_context(tc.tile_pool(name="soft", bufs=4))
            out_pool = ctx.enter_context(tc.tile_pool(name="outp", bufs=2))

            def epilogue(b, psc):
                jmax = int(jmaxs[b])
                nch = nchs[b]
                lns = [min(128, jmax - jc * 128) for jc in range(nch)]
                eT = [
                    soft_pool.tile([128, IB], bf16, tag="eT", name=f"eT{b}_{jc}")
                    for jc in range(nch)
                ]
                for jc in range(nch):
                    nc.scalar.activation(
                        eT[jc][: lns[jc], :],
                        psc[jc][: lns[jc], :],
                        AF.Exp,
                        bias=maskT_sb[: lns[jc], moff[b] + jc:moff[b] + jc + 1],
                    )
                psums = sm_psum.tile([1, IB], f32, tag="sm", name=f"psums{b}")
                for jc in range(nch):
                    nc.tensor.matmul(
                        psums[0:1, :],
                        ones_bf[: lns[jc], 0:1],
                        eT[jc][: lns[jc], :],
                        start=(jc == 0),
                        stop=(jc == nch - 1),
                    )
                rs = soft_pool.tile([1, IB], f32, tag="rs", name=f"rs{b}")
                nc.vector.reciprocal(rs[0:1, :], psums[0:1, :])
                prt = sm_psum.tile([IB, 1], f32, tag="sm", name=f"prt{b}")
                nc.tensor.matmul(
                    prt[:, 0:1], rs[0:1, :], ones_sb[0:1, 0:1], start=True, stop=True
                )
                rinv = soft_pool.tile([IB, 1], f32, tag="rinv", name=f"rinv{b}")
                nc.vector.tensor_copy(rinv[:], prt[:])

                pout = o_psum.tile([IB, V], f32, tag="pout", name=f"pout{b}")
                for jc in range(nch):
                    nc.tensor.matmul(
                        pout[:],
                        eT[jc][: lns[jc], :],
                        values_sb[: lns[jc], (voff[b] // 128 + jc) * V:(voff[b] // 128 + jc + 1) * V],
                        start=(jc == 0),
                        stop=(jc == nch - 1),
                    )
                out_sb = out_pool.tile([IB, V], f32, tag="osb", name=f"osb{b}")
                nc.vector.tensor_scalar_mul(out_sb[:], pout[:], rinv[:])
                nc.sync.dma_start(out=d_out[b * IB:(b + 1) * IB, :], in_=out_sb[:])

            pending = None  # (b, psc) whose epilogue is deferred one batch
            for b in order:
                jmax = int(jmaxs[b])
                jo = int(joff[b])
                nch = nchs[b]
                lns = [min(128, jmax - jc * 128) for jc in range(nch)]
                RB = _row_block(IB, jmax, last=(b == order[-1]))

                psc = [
                    sc_psum.tile([128, IB], f32, tag="pscT", name=f"pscT{b}_{jc}")
                    for jc in range(nch)
                ]
                for r0 in range(0, IB, RB):
                    S = [
                        s_pool.tile(
                            [128, RB * jmax], bf16, tag="S", name=f"S{b}_{r0}_{hcx}"
                        )
                        for hcx in range(HC)
                    ]
                    F = [
                        s_pool.tile(
                            [128, RB * jmax], bf16, tag="F", name=f"F{b}_{r0}_{hcx}"
                        )
                        for hcx in range(HC)
                    ]
                    for hc in range(HC):
                        if jmax <= 128:
                            # narrow window: one broadcast tensor-tensor add
                            # covers the whole row block (per-call DVE
                            # overhead would dominate row-by-row adds)
                            kb = k_sb[hc][:, jo:jo + jmax]
                            k_rep = bass.AP(
                                kb.tensor, kb.offset, [kb.ap[0], [0, RB], kb.ap[1]]
                            )
                            qb = q_sb[hc][:, b * IB + r0:b * IB + r0 + RB]
                            q_rep = bass.AP(
                                qb.tensor, qb.offset, [qb.ap[0], qb.ap[1], [0, jmax]]
                            )
                            sb = S[hc][:, : RB * jmax]
                            s3 = bass.AP(
                                sb.tensor, sb.offset, [sb.ap[0], [jmax, RB], [1, jmax]]
                            )
                            nc.vector.tensor_add(s3, k_rep, q_rep)
                        else:
                            for m in range(RB):
                                row = b * IB + r0 + m
                                nc.vector.tensor_scalar_add(
                                    S[hc][:, m * jmax:(m + 1) * jmax],
                                    k_sb[hc][:, jo:jo + jmax],
                                    q_sb[hc][:, row:row + 1],
                                )
                        # bf16 tanh output: full-128-col weights trigger the
                        # compiler-automatic FWL fast-weight-load path
                        nc.scalar.activation(F[hc][:], S[hc][:], AF.Tanh)
                    for m in range(RB):
                        for jc in range(nch):
                            for hc in range(HC):
                                nc.tensor.matmul(
                                    psc[jc][: lns[jc], r0 + m:r0 + m + 1],
                                    F[hc][:, m * jmax + jc * 128:m * jmax + jc * 128 + lns[jc]],
                                    wv_bf[:, hc:hc + 1],
                                    start=(hc == 0),
                                    stop=(hc == HC - 1),
                                )
                    if r0 == 0 and pending is not None:
                        epilogue(*pending)
                        pending = None
                pending = (b, psc)
            epilogue(*pending)

    _split_multi_waits(nc)
    return nc


def kernel(queries, keys, values, valid_lens, W_q, W_k, w_v):
    global LAST_RESULT
    _install_axon_profile_hook()
    _patch_tile_drain()
    from concourse.bass_utils import run_bass_kernel_spmd

    import ml_dtypes

    bf = ml_dtypes.bfloat16
    queries = np.ascontiguousarray(queries, dtype=np.float32)
    keys = np.ascontiguousarray(keys, dtype=np.float32)
    values = np.ascontiguousarray(values, dtype=np.float32)
    W_q = np.ascontiguousarray(W_q, dtype=np.float32)
    W_k = np.ascontiguousarray(W_k, dtype=np.float32)
    w_v = np.ascontiguousarray(w_v, dtype=np.float32)
    vl = np.asarray(valid_lens).astype(np.int64)

    B, Q, D = queries.shape
    KV = keys.shape[1]
    V = values.shape[2]
    H = W_q.shape[1]
    IB = Q // NCORES
    HC = H // 128

    jmaxs = [min(KV, _ceil_to(max(int(v), 1), 32)) for v in vl]
    jpads = [_ceil_to(j, 128) for j in jmaxs]
    nchs = [j // 128 for j in jpads]
    VTOT = int(np.sum(jpads))

    nc = _build_program(B, Q, D, KV, V, H, jmaxs, IB)

    # ---- shared (core-independent) arrays
    keysT = np.concatenate(
        [keys[b, : jmaxs[b], :].T for b in range(B)], axis=1
    ).astype(bf)  # (D, JTOT)
    values_p = np.zeros((VTOT, V), bf)
    off = 0
    for b in range(B):
        values_p[off:off + jmaxs[b]] = values[b, : jmaxs[b], :].astype(bf)
        off += jpads[b]
    wv2 = w_v.reshape(HC, 128).T.copy()  # (128, HC)
    # additive mask in the transposed layout: one 128-long column per
    # (batch, j-chunk); row p of column (b, jc) corresponds to key j = jc*128+p
    mcols = []
    for b in range(B):
        for jc in range(nchs[b]):
            j = jc * 128 + np.arange(128)
            mcols.append(np.where(j < int(vl[b]), 0.0, NEG).astype(np.float32))
    maskT = np.stack(mcols, axis=1)  # (128, NCHTOT)

    in_maps = []
    for c in range(NCORES):
        queriesT = np.concatenate(
            [queries[b, c * IB:(c + 1) * IB, :].T for b in range(B)], axis=1
        )  # (D, B*IB)
        in_maps.append(
            {
                "queriesT": np.ascontiguousarray(queriesT.astype(bf)),
                "keysT": np.ascontiguousarray(keysT),
                "values_p": values_p,
                "W_q": W_q.astype(bf),
                "W_k": W_k.astype(bf),
                "wv2": wv2,
                "maskT": maskT,
            }
        )

    res = run_bass_kernel_spmd(
        nc, in_maps, core_ids=list(range(NCORES)), trace=TRACE
    )
    LAST_RESULT = res

    out = np.empty((B, Q, V), np.float32)
    for c in range(NCORES):
        o = res.results[c]["out"]  # (B*IB, V)
        for b in range(B):
            out[b, c * IB:(c + 1) * IB, :] = o[b * IB:(b + 1) * IB, :]
    return out


# revision 22
# speedup vs baseline: 1.0443x; 1.0053x over previous
"""Additive attention (B=4, Q=KV=512, H=256) on 8 Trainium2 NeuronCores.

Math (per batch b):
  q = queries @ W_q            (Q, H)
  k = keys    @ W_k            (KV, H)
  scores[i,j] = sum_h w_v[h] * tanh(q[i,h] + k[j,h])
  attn = softmax_j(scores masked to j < valid_lens[b])
  out  = attn @ values         (Q, V)

Sharding: every core takes query rows [c*64, (c+1)*64) of EVERY batch.
That keeps all 8 cores perfectly balanced and the SPMD program uniform even
though the per-batch key window (truncated to ceil(valid/32)*32 columns --
masked columns contribute exactly 0 after softmax) differs per batch.

Device layout: h on partitions for the tanh stage.  For each query row i,
S[h, j] = k[h, j] + q[h, i] is one DVE tensor_scalar_add (per-partition
scalar broadcast); tanh runs in-place on ScalarE over row-blocks.  The
w_v-weighted reduction over h produces scores TRANSPOSED -- for each
(row, 128-wide j-chunk, h-half) one TensorE matmul with the tanh tile as
stationary and the w_v column as the moving operand writes scores_T[j, i]
into PSUM (partition base 0, always legal).  Softmax then works in the
transposed layout: exp(x + mask) is a single ScalarE activation with the
additive mask as per-partition bias, row sums come from a ones-vector
matmul, and the unnormalized exp_T feeds the final values matmul directly
as lhsT (no attention transpose at all); the 1/sum scale is applied to the
output rows as a per-partition DVE scale.
"""

import sys
import types

import numpy as np

NEG = -1.0e6
NCORES = 8
TRACE = False  # test.py flips this to get a profiled run
LAST_RESULT = None  # BassKernelResults stash for test.py


def _install_axon_profile_hook():
    """antenv.axon_hooks is missing from this image; concourse needs it for
    trace=True under axon. Register the ctypes-based NTFF hook manually."""
    import antenv

    if "antenv.axon_hooks" in sys.modules:
        return
    m = types.ModuleType("antenv.axon_hooks")
    m._hook = None

    def _set(h):
        m._hook = h

    def _get():
        return m._hook

    m.set_axon_ntff_profile_hook = _set
    m.get_axon_ntff_profile_hook = _get
    sys.modules["antenv.axon_hooks"] = m
    antenv.axon_hooks = m
    try:
        from trn_agent_boot.trn_boot import _ntff_profile_via_ctypes

        m.set_axon_ntff_profile_hook(
            _ntff_profile_via_ctypes("/opt/axon/libaxon_pjrt.so")
        )
    except Exception:
        pass


def _patch_tile_drain():
    """The walrus build in this image allows at most ONE sync-wait command
    per instruction; Tile's kernel-tail drain carries every vector-clock
    wait on a single drain. Split them across a chain of drains."""
    import concourse.mybir as mybir
    import concourse.tile as tile
    from concourse.vector_clock import ScopedClock

    if getattr(tile.TileContext, "_drain_patched", False):
        return

    def _drain_and_barrier_chunked(self, tick_clock, wait_clock):
        d0 = self.nc.sync.drain()
        wait_clock.add_sem_waits(d0.ins, ScopedClock({None: tick_clock.global_clock}))
        si = d0.ins.sync_info
        waits = list(si.on_wait) if si is not None else []
        if len(waits) > 1:
            # spread the waits round-robin over all engine streams (each
            # instruction may carry at most one wait for this walrus; a
            # serial SP chain would cost ~27 x wait-resolve latency). The
            # all_engine_barrier right after makes the join equivalent.
            engs = [
                mybir.EngineType.SP,
                mybir.EngineType.DVE,
                mybir.EngineType.Activation,
                mybir.EngineType.PE,
                mybir.EngineType.Pool,
            ]
            d0.ins.sync_info = mybir.SyncInfo(
                on_wait=waits[:1], on_update=list(si.on_update)
            )
            for i in range(1, len(waits)):
                ev = mybir.InstEventSemaphore(
                    name=f"tail-wait-{i}",
                    engine=engs[i % len(engs)],
                    ins=[],
                    outs=[],
                    sync_info=mybir.SyncInfo(on_wait=[waits[i]], on_update=[]),
                )
                self.nc.register_instruction(ev)
                self.nc.cur_bb.bb.add_instruction(ev)

        self.nc.all_engine_barrier()
        assert self.sems is not None
        popped = self.nc._tile_sem_poison_stack.pop()
        assert popped is self._sem_poison
        self.nc.clear_and_free_semaphores(list(self.sems.allocated().values()))
        self.nc.all_engine_barrier()

    tile.TileContext._drain_and_barrier = _drain_and_barrier_chunked
    tile.TileContext._drain_patched = True


def _split_multi_waits(nc):
    """walrus here allows one sync-wait command per instruction; move extra
    waits onto standalone EventSemaphore instructions (same engine, just
    before the original instruction -- semantically identical since waits
    are monotonic sem-ge conditions)."""
    import concourse.mybir as mybir

    n = 0
    for fn in nc.m.functions:
        for blk in fn.blocks:
            out = []
            for inst in blk.instructions:
                si = inst.sync_info
                waits = list(si.on_wait) if si is not None else []
                if len(waits) > 1:
                    for k, w in enumerate(waits[:-1]):
                        ev = mybir.InstEventSemaphore(
                            name=f"{inst.name}-xw{k}",
                            engine=inst.engine,
                            ins=[],
                            outs=[],
                            sync_info=mybir.SyncInfo(on_wait=[w], on_update=[]),
                        )
                        out.append(ev)
                        n += 1
                    inst.sync_info = mybir.SyncInfo(
                        on_wait=[waits[-1]], on_update=list(si.on_update)
                    )
                out.append(inst)
            blk.instructions = out
    return n


def _ceil_to(x, m):
    return -(-int(x) // m) * m


def _row_block(IB, jmax, last=False):
    """Rows per tanh block: keep ACT calls ~2-6K elems/lane (divisor of IB).
    Capped at 16 rows so the DVE->ACT->PE pipeline stays fine-grained."""
    rb = max(1, min(IB, 16, 6144 // jmax))
    return 1 << (rb.bit_length() - 1)


def _build_program(B, Q, D, KV, V, H, jmaxs, IB):
    """One Bass program, shared by all 8 cores (SPMD; data differs per core).

    jmaxs[b]: truncated key-window width for batch b (multiple of 32).
    IB: query rows per (core, batch) = Q // NCORES.
    """
    import contextlib

    import concourse.bass as bass
    import concourse.mybir as mybir
    import concourse.tile as tile

    f32 = mybir.dt.float32
    bf16 = mybir.dt.bfloat16
    AF = mybir.ActivationFunctionType

    JTOT = int(np.sum(jmaxs))
    joff = np.concatenate([[0], np.cumsum(jmaxs)]).astype(int)  # key-col offsets
    # values are packed per batch at 128-row boundaries (slot layout)
    jpads = [_ceil_to(j, 128) for j in jmaxs]
    voff = np.concatenate([[0], np.cumsum(jpads)]).astype(int)
    VTOT = int(voff[-1])
    nchs = [_ceil_to(j, 128) // 128 for j in jmaxs]  # j-chunks per batch
    moff = np.concatenate([[0], np.cumsum(nchs)]).astype(int)  # maskT col offsets
    NCHTOT = int(moff[-1])
    NQROWS = B * IB  # query rows per core
    DC = D // 128  # contraction chunks for the projections
    HC = H // 128  # h-halves

    # processing order: widest batch first, narrowest last -- the epilogues
    # are software-pipelined one batch behind, so the tail is the (short)
    # last batch's epilogue chain.
    order = list(np.argsort(jmaxs))[::-1]

    nc = bass.Bass("TRN2", target_bir_lowering=False)
    d_queriesT = nc.declare_dram_parameter("queriesT", [D, NQROWS], bf16, isOutput=False)
    d_keysT = nc.declare_dram_parameter("keysT", [D, JTOT], bf16, isOutput=False)
    d_values = nc.declare_dram_parameter("values_p", [VTOT, V], bf16, isOutput=False)
    d_wq = nc.declare_dram_parameter("W_q", [D, H], bf16, isOutput=False)
    d_wk = nc.declare_dram_parameter("W_k", [D, H], bf16, isOutput=False)
    d_wv = nc.declare_dram_parameter("wv2", [128, HC], f32, isOutput=False)
    d_maskT = nc.declare_dram_parameter("maskT", [128, NCHTOT], f32, isOutput=False)
    d_out = nc.declare_dram_parameter("out", [NQROWS, V], f32, isOutput=True)

    with tile.TileContext(nc) as tc:
        ctx = contextlib.ExitStack()
        with ctx:
            const_pool = ctx.enter_context(tc.tile_pool(name="const", bufs=1))
            w_pool = ctx.enter_context(tc.tile_pool(name="w", bufs=1))
            in_pool = ctx.enter_context(tc.tile_pool(name="in", bufs=1))
            proj_pool = ctx.enter_context(tc.tile_pool(name="proj", bufs=1))

            wv_sb = const_pool.tile([128, HC], f32)
            nc.gpsimd.dma_start(out=wv_sb[:], in_=d_wv[:])
            wv_bf = const_pool.tile([128, HC], bf16)
            nc.vector.tensor_copy(wv_bf[:], wv_sb[:])
            maskT_sb = const_pool.tile([128, NCHTOT], f32)
            nc.gpsimd.dma_start(out=maskT_sb[:], in_=d_maskT[:])
            ones_sb = const_pool.tile([128, 1], f32)
            nc.gpsimd.memset(ones_sb[:], 1.0)
            ones_bf = const_pool.tile([128, 1], bf16)
            nc.gpsimd.memset(ones_bf[:], 1.0)
            warm = const_pool.tile([1, 1], f32)
            nc.scalar.activation(warm[0:1, 0:1], ones_sb[0:1, 0:1], AF.Tanh)

            kT_all = in_pool.tile([128, DC * JTOT], bf16, tag="kT", name="kT_all")
            wq_all = w_pool.tile([128, DC * H], bf16, tag="wq", name="wq_all")
            wk_all = w_pool.tile([128, DC * H], bf16, tag="wk", name="wk_all")
            qT_all = in_pool.tile([128, DC * NQROWS], bf16, tag="qT", name="qT_all")

            def kt_3d(jo, jm):
                base = kT_all[:]
                return bass.AP(
                    base.tensor, base.offset + jo, [base.ap[0], [JTOT, DC], [1, jm]]
                )

            # one wide DMA per logical tensor: per-tensor 3D access patterns
            # put the dc-chunks side by side in SBUF; the serial ~600ns
            # per-dma_start sequencer issue cost was dominating the head.
            b0p = order[0]
            nc.sync.dma_start(
                out=kt_3d(int(joff[b0p]), int(jmaxs[b0p])),
                in_=d_keysT[:, joff[b0p]:joff[b0p] + jmaxs[b0p]].rearrange(
                    "(dc p) j -> p dc j", p=128
                ),
            )
            nc.sync.dma_start(
                out=wk_all[:].rearrange("p (dc h) -> p dc h", h=H),
                in_=d_wk.rearrange("(dc p) h -> p dc h", p=128),
            )
            nc.scalar.dma_start(
                out=wq_all[:].rearrange("p (dc h) -> p dc h", h=H),
                in_=d_wq.rearrange("(dc p) h -> p dc h", p=128),
            )
            nc.scalar.dma_start(
                out=qT_all[:].rearrange("p (dc r) -> p dc r", r=NQROWS),
                in_=d_queriesT.rearrange("(dc p) r -> p dc r", p=128),
            )
            for b in order[1:]:
                nc.sync.dma_start(
                    out=kt_3d(int(joff[b]), int(jmaxs[b])),
                    in_=d_keysT[:, joff[b]:joff[b] + jmaxs[b]].rearrange(
                        "(dc p) j -> p dc j", p=128
                    ),
                )

            values_sb = in_pool.tile([128, (VTOT // 128) * V], bf16, tag="vals")
            nc.gpsimd.dma_start(
                out=values_sb[:].rearrange("p (s v) -> p s v", v=V),
                in_=d_values.rearrange("(s p) v -> p s v", p=128),
            )

            # ---- projections (per batch window, first-processed first)
            # q/k slabs feed the DVE broadcast-add: k in bf16 (4x DVE mode),
            # q stays f32 (tensor_scalar scalar operand must be f32)
            q_sb = [
                proj_pool.tile([128, NQROWS], f32, tag=f"q{hc}", name=f"qsb{hc}")
                for hc in range(HC)
            ]
            k_sb = [
                proj_pool.tile([128, JTOT], bf16, tag=f"k{hc}", name=f"ksb{hc}")
                for hc in range(HC)
            ]
            if True:
                ppsum = ctx.enter_context(tc.tile_pool(name="ppsum", bufs=1, space="PSUM"))
                def proj_k(b, hcs=None):
                    jo, jm = int(joff[b]), int(jmaxs[b])
                    for hc in hcs if hcs is not None else range(HC):
                        pk = ppsum.tile([128, 512], f32, tag="pproj", name="pk")
                        for dc in range(DC):
                            nc.tensor.matmul(
                                pk[:, :jm],
                                wk_all[:, dc * H + hc * 128:dc * H + (hc + 1) * 128],
                                kT_all[:, dc * JTOT + jo:dc * JTOT + jo + jm],
                                start=(dc == 0),
                                stop=(dc == DC - 1),
                            )
                        nc.vector.tensor_copy(k_sb[hc][:, jo:jo + jm], pk[:, :jm])

                def proj_q(hc):
                    pq = ppsum.tile([128, NQROWS], f32, tag="pproj", name="pq")
                    for dc in range(DC):
                        nc.tensor.matmul(
                            pq[:],
                            wq_all[:, dc * H + hc * 128:dc * H + (hc + 1) * 128],
                            qT_all[:, dc * NQROWS:(dc + 1) * NQROWS],
                            start=(dc == 0),
                            stop=(dc == DC - 1),
                        )
                    nc.vector.tensor_copy(q_sb[hc][:], pq[:])

                proj_k(order[0], hcs=[0])
                proj_q(0)
                proj_k(order[0], hcs=list(range(1, HC)))
                for hc in range(1, HC):
                    proj_q(hc)
                late_projs = [lambda b=b: proj_k(b) for b in order[1:]]

            # ---- main: tanh features -> transposed scores -> softmax -> out
            s_pool = ctx.enter_context(tc.tile_pool(name="S", bufs=8))
            sc_psum = ctx.enter_context(tc.tile_pool(name="scps", bufs=5, space="PSUM"))
            sm_psum = ctx.enter_context(tc.tile_pool(name="smps", bufs=1, space="PSUM"))
            o_psum = ctx.enter_context(tc.tile_pool(name="ops", bufs=1, space="PSUM"))
            soft_pool = ctx.enter_context(tc.tile_pool(name="soft", bufs=4))
            out_pool = ctx.enter_context(tc.tile_pool(name="outp", bufs=2))

            def epilogue(b, psc):
                jmax = int(jmaxs[b])
                nch = nchs[b]
                lns = [min(128, jmax - jc * 128) for jc in range(nch)]
                eT = [
                    soft_pool.tile([128, IB], bf16, tag="eT", name=f"eT{b}_{jc}")
                    for jc in range(nch)
                ]
                for jc in range(nch):
                    nc.scalar.activation(
                        eT[jc][: lns[jc], :],
                        psc[jc][: lns[jc], :],
                        AF.Exp,
                        bias=maskT_sb[: lns[jc], moff[b] + jc:moff[b] + jc + 1],
                    )
                psums = sm_psum.tile([1, IB], f32, tag="sm", name=f"psums{b}")
                for jc in range(nch):
                    nc.tensor.matmul(
                        psums[0:1, :],
                        ones_bf[: lns[jc], 0:1],
                        eT[jc][: lns[jc], :],
                        start=(jc == 0),
                        stop=(jc == nch - 1),
                    )
                rs = soft_pool.tile([1, IB], f32, tag="rs", name=f"rs{b}")
                nc.vector.reciprocal(rs[0:1, :], psums[0:1, :])
                prt = sm_psum.tile([IB, 1], f32, tag="sm", name=f"prt{b}")
                nc.tensor.matmul(
                    prt[:, 0:1], rs[0:1, :], ones_sb[0:1, 0:1], start=True, stop=True
                )
                rinv = soft_pool.tile([IB, 1], f32, tag="rinv", name=f"rinv{b}")
                nc.vector.tensor_copy(rinv[:], prt[:])

                pout = o_psum.tile([IB, V], f32, tag="pout", name=f"pout{b}")
                for jc in range(nch):
                    nc.tensor.matmul(
                        pout[:],
                        eT[jc][: lns[jc], :],
                        values_sb[: lns[jc], (voff[b] // 128 + jc) * V:(voff[b] // 128 + jc + 1) * V],
                        start=(jc == 0),
                        stop=(jc == nch - 1),
                    )
                out_sb = out_pool.tile([IB, V], f32, tag="osb", name=f"osb{b}")
                nc.vector.tensor_scalar_mul(out_sb[:], pout[:], rinv[:])
                nc.sync.dma_start(out=d_out[b * IB:(b + 1) * IB, :], in_=out_sb[:])

            pending = None  # (b, psc) whose epilogue is deferred one batch
            for b in order:
                jmax = int(jmaxs[b])
                jo = int(joff[b])
                nch = nchs[b]
                lns = [min(128, jmax - jc * 128) for jc in range(nch)]
                RB = _row_block(IB, jmax, last=(b == order[-1]))

                psc = [
                    sc_psum.tile([128, IB], f32, tag="pscT", name=f"pscT{b}_{jc}")
                    for jc in range(nch)
                ]
                blocks = []
                r = 0
                while r < IB:
                    if r + RB >= IB and RB > 8:
                        blocks += [(r, RB // 2), (r + RB // 2, RB - RB // 2)]
                        r += RB
                    else:
                        blocks.append((r, RB))
                        r += RB
                for r0, rb in blocks:
                    S = [
                        s_pool.tile(
                            [128, rb * jmax], bf16, tag="S", name=f"S{b}_{r0}_{hcx}"
                        )
                        for hcx in range(HC)
                    ]
                    F = [
                        s_pool.tile(
                            [128, rb * jmax], bf16, tag="F", name=f"F{b}_{r0}_{hcx}"
                        )
                        for hcx in range(HC)
                    ]
                    for hc in range(HC):
                        if jmax <= 128:
                            # narrow window: one broadcast tensor-tensor add
                            # covers the whole row block (per-call DVE
                            # overhead would dominate row-by-row adds)
                            kb = k_sb[hc][:, jo:jo + jmax]
                            k_rep = bass.AP(
                                kb.tensor, kb.offset, [kb.ap[0], [0, rb], kb.ap[1]]
                            )
                            qb = q_sb[hc][:, b * IB + r0:b * IB + r0 + rb]
                            q_rep = bass.AP(
                                qb.tensor, qb.offset, [qb.ap[0], qb.ap[1], [0, jmax]]
                            )
                            sb = S[hc][:, : rb * jmax]
                            s3 = bass.AP(
                                sb.tensor, sb.offset, [sb.ap[0], [jmax, rb], [1, jmax]]
                            )
                            nc.vector.tensor_add(s3, k_rep, q_rep)
                        else:
                            for m in range(rb):
                                row = b * IB + r0 + m
                                nc.vector.tensor_scalar_add(
                                    S[hc][:, m * jmax:(m + 1) * jmax],
                                    k_sb[hc][:, jo:jo + jmax],
                                    q_sb[hc][:, row:row + 1],
                                )
                        # bf16 tanh output: full-128-col weights trigger the
                        # compiler-automatic FWL fast-weight-load path
                        nc.scalar.activation(F[hc][:], S[hc][:], AF.Tanh)
                    for m in range(rb):
                        for jc in range(nch):
                            for hc in range(HC):
                                nc.tensor.matmul(
                                    psc[jc][: lns[jc], r0 + m:r0 + m + 1],
                                    F[hc][:, m * jmax + jc * 128:m * jmax + jc * 128 + lns[jc]],
                                    wv_bf[:, hc:hc + 1],
                                    start=(hc == 0),
                                    stop=(hc == HC - 1),
                                )
                    if r0 == 0:
                        if pending is not None:
                            epilogue(*pending)
                            pending = None
                        while late_projs:
                            late_projs.pop(0)()
                pending = (b, psc)
            epilogue(*pending)

    _split_multi_waits(nc)
    return nc


def kernel(queries, keys, values, valid_lens, W_q, W_k, w_v):
    global LAST_RESULT
    _install_axon_profile_hook()
    _patch_tile_drain()
    from concourse.bass_utils import run_bass_kernel_spmd

    import ml_dtypes

    bf = ml_dtypes.bfloat16
    queries = np.ascontiguousarray(queries, dtype=np.float32)
    keys = np.ascontiguousarray(keys, dtype=np.float32)
    values = np.ascontiguousarray(values, dtype=np.float32)
    W_q = np.ascontiguousarray(W_q, dtype=np.float32)
    W_k = np.ascontiguousarray(W_k, dtype=np.float32)
    w_v = np.ascontiguousarray(w_v, dtype=np.float32)
    vl = np.asarray(valid_lens).astype(np.int64)

    B, Q, D = queries.shape
    KV = keys.shape[1]
    V = values.shape[2]
    H = W_q.shape[1]
    IB = Q // NCORES
    HC = H // 128

    jmaxs = [min(KV, _ceil_to(max(int(v), 1), 32)) for v in vl]
    jpads = [_ceil_to(j, 128) for j in jmaxs]
    nchs = [j // 128 for j in jpads]
    VTOT = int(np.sum(jpads))

    nc = _build_program(B, Q, D, KV, V, H, jmaxs, IB)

    # ---- shared (core-independent) arrays
    keysT = np.concatenate(
        [keys[b, : jmaxs[b], :].T for b in range(B)], axis=1
    ).astype(bf)  # (D, JTOT)
    values_p = np.zeros((VTOT, V), bf)
    off = 0
    for b in range(B):
        values_p[off:off + jmaxs[b]] = values[b, : jmaxs[b], :].astype(bf)
        off += jpads[b]
    wv2 = w_v.reshape(HC, 128).T.copy()  # (128, HC)
    # additive mask in the transposed layout: one 128-long column per
    # (batch, j-chunk); row p of column (b, jc) corresponds to key j = jc*128+p
    mcols = []
    for b in range(B):
        for jc in range(nchs[b]):
            j = jc * 128 + np.arange(128)
            mcols.append(np.where(j < int(vl[b]), 0.0, NEG).astype(np.float32))
    maskT = np.stack(mcols, axis=1)  # (128, NCHTOT)

    in_maps = []
    for c in range(NCORES):
        queriesT = np.concatenate(
            [queries[b, c * IB:(c + 1) * IB, :].T for b in range(B)], axis=1
        )  # (D, B*IB)
        in_maps.append(
            {
                "queriesT": np.ascontiguousarray(queriesT.astype(bf)),
                "keysT": np.ascontiguousarray(keysT),
                "values_p": values_p,
                "W_q": W_q.astype(bf),
                "W_k": W_k.astype(bf),
                "wv2": wv2,
                "maskT": maskT,
            }
        )

    res = run_bass_kernel_spmd(
        nc, in_maps, core_ids=list(range(NCORES)), trace=TRACE
    )
    LAST_RESULT = res

    out = np.empty((B, Q, V), np.float32)
    for c in range(NCORES):
        o = res.results[c]["out"]  # (B*IB, V)
        for b in range(B):
            out[b, c * IB:(c + 1) * IB, :] = o[b * IB:(b + 1) * IB, :]
    return out


# revision 23
# speedup vs baseline: 1.0475x; 1.0031x over previous
"""Additive attention (B=4, Q=KV=512, H=256) on 8 Trainium2 NeuronCores.

Math (per batch b):
  q = queries @ W_q            (Q, H)
  k = keys    @ W_k            (KV, H)
  scores[i,j] = sum_h w_v[h] * tanh(q[i,h] + k[j,h])
  attn = softmax_j(scores masked to j < valid_lens[b])
  out  = attn @ values         (Q, V)

Sharding: every core takes query rows [c*64, (c+1)*64) of EVERY batch.
That keeps all 8 cores perfectly balanced and the SPMD program uniform even
though the per-batch key window (truncated to ceil(valid/32)*32 columns --
masked columns contribute exactly 0 after softmax) differs per batch.

Device layout: h on partitions for the tanh stage.  For each query row i,
S[h, j] = k[h, j] + q[h, i] is one DVE tensor_scalar_add (per-partition
scalar broadcast); tanh runs in-place on ScalarE over row-blocks.  The
w_v-weighted reduction over h produces scores TRANSPOSED -- for each
(row, 128-wide j-chunk, h-half) one TensorE matmul with the tanh tile as
stationary and the w_v column as the moving operand writes scores_T[j, i]
into PSUM (partition base 0, always legal).  Softmax then works in the
transposed layout: exp(x + mask) is a single ScalarE activation with the
additive mask as per-partition bias, row sums come from a ones-vector
matmul, and the unnormalized exp_T feeds the final values matmul directly
as lhsT (no attention transpose at all); the 1/sum scale is applied to the
output rows as a per-partition DVE scale.
"""

import sys
import types

import numpy as np

NEG = -1.0e6
NCORES = 8
TRACE = False  # test.py flips this to get a profiled run
LAST_RESULT = None  # BassKernelResults stash for test.py


def _install_axon_profile_hook():
    """antenv.axon_hooks is missing from this image; concourse needs it for
    trace=True under axon. Register the ctypes-based NTFF hook manually."""
    import antenv

    if "antenv.axon_hooks" in sys.modules:
        return
    m = types.ModuleType("antenv.axon_hooks")
    m._hook = None

    def _set(h):
        m._hook = h

    def _get():
        return m._hook

    m.set_axon_ntff_profile_hook = _set
    m.get_axon_ntff_profile_hook = _get
    sys.modules["antenv.axon_hooks"] = m
    antenv.axon_hooks = m
    try:
        from trn_agent_boot.trn_boot import _ntff_profile_via_ctypes

        m.set_axon_ntff_profile_hook(
            _ntff_profile_via_ctypes("/opt/axon/libaxon_pjrt.so")
        )
    except Exception:
        pass


def _patch_tile_drain():
    """The walrus build in this image allows at most ONE sync-wait command
    per instruction; Tile's kernel-tail drain carries every vector-clock
    wait on a single drain. Split them across a chain of drains."""
    import concourse.mybir as mybir
    import concourse.tile as tile
    from concourse.vector_clock import ScopedClock

    if getattr(tile.TileContext, "_drain_patched", False):
        return

    def _drain_and_barrier_chunked(self, tick_clock, wait_clock):
        d0 = self.nc.sync.drain()
        wait_clock.add_sem_waits(d0.ins, ScopedClock({None: tick_clock.global_clock}))
        si = d0.ins.sync_info
        waits = list(si.on_wait) if si is not None else []
        if len(waits) > 1:
            # spread the waits round-robin over all engine streams (each
            # instruction may carry at most one wait for this walrus; a
            # serial SP chain would cost ~27 x wait-resolve latency). The
            # all_engine_barrier right after makes the join equivalent.
            engs = [
                mybir.EngineType.SP,
                mybir.EngineType.DVE,
                mybir.EngineType.Activation,
                mybir.EngineType.PE,
                mybir.EngineType.Pool,
            ]
            d0.ins.sync_info = mybir.SyncInfo(
                on_wait=waits[:1], on_update=list(si.on_update)
            )
            for i in range(1, len(waits)):
                ev = mybir.InstEventSemaphore(
                    name=f"tail-wait-{i}",
                    engine=engs[i % len(engs)],
                    ins=[],
                    outs=[],
                    sync_info=mybir.SyncInfo(on_wait=[waits[i]], on_update=[]),
                )
                self.nc.register_instruction(ev)
                self.nc.cur_bb.bb.add_instruction(ev)

        self.nc.all_engine_barrier()
        assert self.sems is not None
        popped = self.nc._tile_sem_poison_stack.pop()
        assert popped is self._sem_poison
        self.nc.clear_and_free_semaphores(list(self.sems.allocated().values()))
        self.nc.all_engine_barrier()

    tile.TileContext._drain_and_barrier = _drain_and_barrier_chunked
    tile.TileContext._drain_patched = True


def _split_multi_waits(nc):
    """walrus here allows one sync-wait command per instruction; move extra
    waits onto standalone EventSemaphore instructions (same engine, just
    before the original instruction -- semantically identical since waits
    are monotonic sem-ge conditions)."""
    import concourse.mybir as mybir

    n = 0
    for fn in nc.m.functions:
        for blk in fn.blocks:
            out = []
            for inst in blk.instructions:
                si = inst.sync_info
                waits = list(si.on_wait) if si is not None else []
                if len(waits) > 1:
                    for k, w in enumerate(waits[:-1]):
                        ev = mybir.InstEventSemaphore(
                            name=f"{inst.name}-xw{k}",
                            engine=inst.engine,
                            ins=[],
                            outs=[],
                            sync_info=mybir.SyncInfo(on_wait=[w], on_update=[]),
                        )
                        out.append(ev)
                        n += 1
                    inst.sync_info = mybir.SyncInfo(
                        on_wait=[waits[-1]], on_update=list(si.on_update)
                    )
                out.append(inst)
            blk.instructions = out
    return n


def _ceil_to(x, m):
    return -(-int(x) // m) * m


def _row_block(IB, jmax, last=False):
    """Rows per tanh block: keep ACT calls ~2-6K elems/lane (divisor of IB).
    Capped at 16 rows so the DVE->ACT->PE pipeline stays fine-grained."""
    rb = max(1, min(IB, 16, 6144 // jmax))
    return 1 << (rb.bit_length() - 1)


def _build_program(B, Q, D, KV, V, H, jmaxs, IB):
    """One Bass program, shared by all 8 cores (SPMD; data differs per core).

    jmaxs[b]: truncated key-window width for batch b (multiple of 32).
    IB: query rows per (core, batch) = Q // NCORES.
    """
    import contextlib

    import concourse.bass as bass
    import concourse.mybir as mybir
    import concourse.tile as tile

    f32 = mybir.dt.float32
    bf16 = mybir.dt.bfloat16
    AF = mybir.ActivationFunctionType

    JTOT = int(np.sum(jmaxs))
    joff = np.concatenate([[0], np.cumsum(jmaxs)]).astype(int)  # key-col offsets
    # values are packed per batch at 128-row boundaries (slot layout)
    jpads = [_ceil_to(j, 128) for j in jmaxs]
    voff = np.concatenate([[0], np.cumsum(jpads)]).astype(int)
    VTOT = int(voff[-1])
    nchs = [_ceil_to(j, 128) // 128 for j in jmaxs]  # j-chunks per batch
    moff = np.concatenate([[0], np.cumsum(nchs)]).astype(int)  # maskT col offsets
    NCHTOT = int(moff[-1])
    NQROWS = B * IB  # query rows per core
    DC = D // 128  # contraction chunks for the projections
    HC = H // 128  # h-halves

    # processing order: widest batch first, narrowest last -- the epilogues
    # are software-pipelined one batch behind, so the tail is the (short)
    # last batch's epilogue chain.
    order = list(np.argsort(jmaxs))[::-1]

    nc = bass.Bass("TRN2", target_bir_lowering=False)
    d_queriesT = nc.declare_dram_parameter("queriesT", [D, NQROWS], bf16, isOutput=False)
    d_keysT = nc.declare_dram_parameter("keysT", [D, JTOT], bf16, isOutput=False)
    d_values = nc.declare_dram_parameter("values_p", [VTOT, V], bf16, isOutput=False)
    d_wq = nc.declare_dram_parameter("W_q", [D, H], bf16, isOutput=False)
    d_wk = nc.declare_dram_parameter("W_k", [D, H], bf16, isOutput=False)
    d_wv = nc.declare_dram_parameter("wv2", [128, HC], f32, isOutput=False)
    d_maskT = nc.declare_dram_parameter("maskT", [128, NCHTOT], f32, isOutput=False)
    d_out = nc.declare_dram_parameter("out", [NQROWS, V], f32, isOutput=True)

    with tile.TileContext(nc) as tc:
        ctx = contextlib.ExitStack()
        with ctx:
            const_pool = ctx.enter_context(tc.tile_pool(name="const", bufs=1))
            w_pool = ctx.enter_context(tc.tile_pool(name="w", bufs=1))
            in_pool = ctx.enter_context(tc.tile_pool(name="in", bufs=1))
            proj_pool = ctx.enter_context(tc.tile_pool(name="proj", bufs=1))

            wv_sb = const_pool.tile([128, HC], f32)
            nc.gpsimd.dma_start(out=wv_sb[:], in_=d_wv[:])
            wv_bf = const_pool.tile([128, HC], bf16)
            nc.vector.tensor_copy(wv_bf[:], wv_sb[:])
            maskT_sb = const_pool.tile([128, NCHTOT], f32)
            nc.gpsimd.dma_start(out=maskT_sb[:], in_=d_maskT[:])
            ones_sb = const_pool.tile([128, 1], f32)
            nc.gpsimd.memset(ones_sb[:], 1.0)
            ones_bf = const_pool.tile([128, 1], bf16)
            nc.gpsimd.memset(ones_bf[:], 1.0)
            warm = const_pool.tile([1, 1], f32)
            nc.scalar.activation(warm[0:1, 0:1], ones_sb[0:1, 0:1], AF.Tanh)

            kT_all = in_pool.tile([128, DC * JTOT], bf16, tag="kT", name="kT_all")
            wq_all = w_pool.tile([128, DC * H], bf16, tag="wq", name="wq_all")
            wk_all = w_pool.tile([128, DC * H], bf16, tag="wk", name="wk_all")
            qT_all = in_pool.tile([128, DC * NQROWS], bf16, tag="qT", name="qT_all")

            def kt_3d(jo, jm):
                base = kT_all[:]
                return bass.AP(
                    base.tensor, base.offset + jo, [base.ap[0], [JTOT, DC], [1, jm]]
                )

            # one wide DMA per logical tensor: per-tensor 3D access patterns
            # put the dc-chunks side by side in SBUF; the serial ~600ns
            # per-dma_start sequencer issue cost was dominating the head.
            b0p = order[0]
            nc.sync.dma_start(
                out=kt_3d(int(joff[b0p]), int(jmaxs[b0p])),
                in_=d_keysT[:, joff[b0p]:joff[b0p] + jmaxs[b0p]].rearrange(
                    "(dc p) j -> p dc j", p=128
                ),
            )
            nc.sync.dma_start(
                out=wk_all[:].rearrange("p (dc h) -> p dc h", h=H),
                in_=d_wk.rearrange("(dc p) h -> p dc h", p=128),
            )
            nc.scalar.dma_start(
                out=wq_all[:].rearrange("p (dc h) -> p dc h", h=H),
                in_=d_wq.rearrange("(dc p) h -> p dc h", p=128),
            )
            nc.scalar.dma_start(
                out=qT_all[:].rearrange("p (dc r) -> p dc r", r=NQROWS),
                in_=d_queriesT.rearrange("(dc p) r -> p dc r", p=128),
            )
            for b in order[1:]:
                nc.sync.dma_start(
                    out=kt_3d(int(joff[b]), int(jmaxs[b])),
                    in_=d_keysT[:, joff[b]:joff[b] + jmaxs[b]].rearrange(
                        "(dc p) j -> p dc j", p=128
                    ),
                )

            values_sb = in_pool.tile([128, (VTOT // 128) * V], bf16, tag="vals")
            nc.gpsimd.dma_start(
                out=values_sb[:].rearrange("p (s v) -> p s v", v=V),
                in_=d_values.rearrange("(s p) v -> p s v", p=128),
            )

            # ---- projections (per batch window, first-processed first)
            # q/k slabs feed the DVE broadcast-add: k in bf16 (4x DVE mode),
            # q stays f32 (tensor_scalar scalar operand must be f32)
            q_sb = [
                proj_pool.tile([128, NQROWS], f32, tag=f"q{hc}", name=f"qsb{hc}")
                for hc in range(HC)
            ]
            k_sb = [
                proj_pool.tile([128, JTOT], bf16, tag=f"k{hc}", name=f"ksb{hc}")
                for hc in range(HC)
            ]
            if True:
                ppsum = ctx.enter_context(tc.tile_pool(name="ppsum", bufs=1, space="PSUM"))
                def proj_k(b, hcs=None):
                    jo, jm = int(joff[b]), int(jmaxs[b])
                    for hc in hcs if hcs is not None else range(HC):
                        pk = ppsum.tile([128, 512], f32, tag="pproj", name="pk")
                        for dc in range(DC):
                            nc.tensor.matmul(
                                pk[:, :jm],
                                wk_all[:, dc * H + hc * 128:dc * H + (hc + 1) * 128],
                                kT_all[:, dc * JTOT + jo:dc * JTOT + jo + jm],
                                start=(dc == 0),
                                stop=(dc == DC - 1),
                            )
                        nc.vector.tensor_copy(k_sb[hc][:, jo:jo + jm], pk[:, :jm])

                def proj_q(hc):
                    pq = ppsum.tile([128, NQROWS], f32, tag="pproj", name="pq")
                    for dc in range(DC):
                        nc.tensor.matmul(
                            pq[:],
                            wq_all[:, dc * H + hc * 128:dc * H + (hc + 1) * 128],
                            qT_all[:, dc * NQROWS:(dc + 1) * NQROWS],
                            start=(dc == 0),
                            stop=(dc == DC - 1),
                        )
                    nc.vector.tensor_copy(q_sb[hc][:], pq[:])

                proj_k(order[0], hcs=[0])
                proj_q(0)
                proj_k(order[0], hcs=list(range(1, HC)))
                for hc in range(1, HC):
                    proj_q(hc)
                late_projs = [lambda b=b: proj_k(b) for b in order[1:]]

            # ---- main: tanh features -> transposed scores -> softmax -> out
            # S/F slot = biggest row-block; keep total S+F pool usage under
            # ~110KB/partition so worst-case valid_lens still fit SBUF
            slot = max(
                _row_block(IB, int(j), last=(bb == order[-1])) * int(j) * 2
                for bb, j in enumerate(jmaxs)
            )
            s_bufs = max(3, min(8, (110 * 1024) // (2 * slot)))
            s_pool = ctx.enter_context(tc.tile_pool(name="S", bufs=s_bufs))
            sc_psum = ctx.enter_context(tc.tile_pool(name="scps", bufs=5, space="PSUM"))
            sm_psum = ctx.enter_context(tc.tile_pool(name="smps", bufs=1, space="PSUM"))
            o_psum = ctx.enter_context(tc.tile_pool(name="ops", bufs=1, space="PSUM"))
            soft_pool = ctx.enter_context(tc.tile_pool(name="soft", bufs=4))
            out_pool = ctx.enter_context(tc.tile_pool(name="outp", bufs=2))

            def epilogue(b, psc):
                jmax = int(jmaxs[b])
                nch = nchs[b]
                lns = [min(128, jmax - jc * 128) for jc in range(nch)]
                eT = [
                    soft_pool.tile([128, IB], bf16, tag="eT", name=f"eT{b}_{jc}")
                    for jc in range(nch)
                ]
                for jc in range(nch):
                    nc.scalar.activation(
                        eT[jc][: lns[jc], :],
                        psc[jc][: lns[jc], :],
                        AF.Exp,
                        bias=maskT_sb[: lns[jc], moff[b] + jc:moff[b] + jc + 1],
                    )
                psums = sm_psum.tile([1, IB], f32, tag="sm", name=f"psums{b}")
                for jc in range(nch):
                    nc.tensor.matmul(
                        psums[0:1, :],
                        ones_bf[: lns[jc], 0:1],
                        eT[jc][: lns[jc], :],
                        start=(jc == 0),
                        stop=(jc == nch - 1),
                    )
                rs = soft_pool.tile([1, IB], f32, tag="rs", name=f"rs{b}")
                nc.vector.reciprocal(rs[0:1, :], psums[0:1, :])
                prt = sm_psum.tile([IB, 1], f32, tag="sm", name=f"prt{b}")
                nc.tensor.matmul(
                    prt[:, 0:1], rs[0:1, :], ones_sb[0:1, 0:1], start=True, stop=True
                )
                rinv = soft_pool.tile([IB, 1], f32, tag="rinv", name=f"rinv{b}")
                nc.vector.tensor_copy(rinv[:], prt[:])

                pout = o_psum.tile([IB, V], f32, tag="pout", name=f"pout{b}")
                for jc in range(nch):
                    nc.tensor.matmul(
                        pout[:],
                        eT[jc][: lns[jc], :],
                        values_sb[: lns[jc], (voff[b] // 128 + jc) * V:(voff[b] // 128 + jc + 1) * V],
                        start=(jc == 0),
                        stop=(jc == nch - 1),
                    )
                out_sb = out_pool.tile([IB, V], f32, tag="osb", name=f"osb{b}")
                nc.vector.tensor_scalar_mul(out_sb[:], pout[:], rinv[:])
                nc.sync.dma_start(out=d_out[b * IB:(b + 1) * IB, :], in_=out_sb[:])

            pending = None  # (b, psc) whose epilogue is deferred one batch
            for b in order:
                jmax = int(jmaxs[b])
                jo = int(joff[b])
                nch = nchs[b]
                lns = [min(128, jmax - jc * 128) for jc in range(nch)]
                RB = _row_block(IB, jmax, last=(b == order[-1]))

                psc = [
                    sc_psum.tile([128, IB], f32, tag="pscT", name=f"pscT{b}_{jc}")
                    for jc in range(nch)
                ]
                blocks = []
                r = 0
                while r < IB:
                    if r + RB >= IB and RB > 8:
                        blocks += [(r, RB // 2), (r + RB // 2, RB - RB // 2)]
                        r += RB
                    else:
                        blocks.append((r, RB))
                        r += RB
                for r0, rb in blocks:
                    S = [
                        s_pool.tile(
                            [128, rb * jmax], bf16, tag="S", name=f"S{b}_{r0}_{hcx}"
                        )
                        for hcx in range(HC)
                    ]
                    F = [
                        s_pool.tile(
                            [128, rb * jmax], bf16, tag="F", name=f"F{b}_{r0}_{hcx}"
                        )
                        for hcx in range(HC)
                    ]
                    for hc in range(HC):
                        if jmax <= 128:
                            # narrow window: one broadcast tensor-tensor add
                            # covers the whole row block (per-call DVE
                            # overhead would dominate row-by-row adds)
                            kb = k_sb[hc][:, jo:jo + jmax]
                            k_rep = bass.AP(
                                kb.tensor, kb.offset, [kb.ap[0], [0, rb], kb.ap[1]]
                            )
                            qb = q_sb[hc][:, b * IB + r0:b * IB + r0 + rb]
                            q_rep = bass.AP(
                                qb.tensor, qb.offset, [qb.ap[0], qb.ap[1], [0, jmax]]
                            )
                            sb = S[hc][:, : rb * jmax]
                            s3 = bass.AP(
                                sb.tensor, sb.offset, [sb.ap[0], [jmax, rb], [1, jmax]]
                            )
                            nc.vector.tensor_add(s3, k_rep, q_rep)
                        else:
                            for m in range(rb):
                                row = b * IB + r0 + m
                                nc.vector.tensor_scalar_add(
                                    S[hc][:, m * jmax:(m + 1) * jmax],
                                    k_sb[hc][:, jo:jo + jmax],
                                    q_sb[hc][:, row:row + 1],
                                )
                        # bf16 tanh output: full-128-col weights trigger the
                        # compiler-automatic FWL fast-weight-load path
                        nc.scalar.activation(F[hc][:], S[hc][:], AF.Tanh)
                    for m in range(rb):
                        for jc in range(nch):
                            for hc in range(HC):
                                nc.tensor.matmul(
                                    psc[jc][: lns[jc], r0 + m:r0 + m + 1],
                                    F[hc][:, m * jmax + jc * 128:m * jmax + jc * 128 + lns[jc]],
                                    wv_bf[:, hc:hc + 1],
                                    start=(hc == 0),
                                    stop=(hc == HC - 1),
                                )
                    if r0 == 0:
                        if pending is not None:
                            epilogue(*pending)
                            pending = None
                        while late_projs:
                            late_projs.pop(0)()
                pending = (b, psc)
            epilogue(*pending)

    _split_multi_waits(nc)
    return nc


def kernel(queries, keys, values, valid_lens, W_q, W_k, w_v):
    global LAST_RESULT
    _install_axon_profile_hook()
    _patch_tile_drain()
    from concourse.bass_utils import run_bass_kernel_spmd

    import ml_dtypes

    bf = ml_dtypes.bfloat16
    queries = np.ascontiguousarray(queries, dtype=np.float32)
    keys = np.ascontiguousarray(keys, dtype=np.float32)
    values = np.ascontiguousarray(values, dtype=np.float32)
    W_q = np.ascontiguousarray(W_q, dtype=np.float32)
    W_k = np.ascontiguousarray(W_k, dtype=np.float32)
    w_v = np.ascontiguousarray(w_v, dtype=np.float32)
    vl = np.asarray(valid_lens).astype(np.int64)

    B, Q, D = queries.shape
    KV = keys.shape[1]
    V = values.shape[2]
    H = W_q.shape[1]
    IB = Q // NCORES
    HC = H // 128

    jmaxs = [min(KV, _ceil_to(max(int(v), 1), 32)) for v in vl]
    jpads = [_ceil_to(j, 128) for j in jmaxs]
    nchs = [j // 128 for j in jpads]
    VTOT = int(np.sum(jpads))

    nc = _build_program(B, Q, D, KV, V, H, jmaxs, IB)

    # ---- shared (core-independent) arrays
    keysT = np.concatenate(
        [keys[b, : jmaxs[b], :].T for b in range(B)], axis=1
    ).astype(bf)  # (D, JTOT)
    values_p = np.zeros((VTOT, V), bf)
    off = 0
    for b in range(B):
        values_p[off:off + jmaxs[b]] = values[b, : jmaxs[b], :].astype(bf)
        off += jpads[b]
    wv2 = w_v.reshape(HC, 128).T.copy()  # (128, HC)
    # additive mask in the transposed layout: one 128-long column per
    # (batch, j-chunk); row p of column (b, jc) corresponds to key j = jc*128+p
    mcols = []
    for b in range(B):
        for jc in range(nchs[b]):
            j = jc * 128 + np.arange(128)
            mcols.append(np.where(j < int(vl[b]), 0.0, NEG).astype(np.float32))
    maskT = np.stack(mcols, axis=1)  # (128, NCHTOT)

    in_maps = []
    for c in range(NCORES):
        queriesT = np.concatenate(
            [queries[b, c * IB:(c + 1) * IB, :].T for b in range(B)], axis=1
        )  # (D, B*IB)
        in_maps.append(
            {
                "queriesT": np.ascontiguousarray(queriesT.astype(bf)),
                "keysT": np.ascontiguousarray(keysT),
                "values_p": values_p,
                "W_q": W_q.astype(bf),
                "W_k": W_k.astype(bf),
                "wv2": wv2,
                "maskT": maskT,
            }
        )

    res = run_bass_kernel_spmd(
        nc, in_maps, core_ids=list(range(NCORES)), trace=TRACE
    )
    LAST_RESULT = res

    out = np.empty((B, Q, V), np.float32)
    for c in range(NCORES):
        o = res.results[c]["out"]  # (B*IB, V)
        for b in range(B):
            out[b, c * IB:(c + 1) * IB, :] = o[b * IB:(b + 1) * IB, :]
    return out


# revision 24
# speedup vs baseline: 1.0562x; 1.0082x over previous
"""Additive attention (B=4, Q=KV=512, H=256) on 8 Trainium2 NeuronCores.

Math (per batch b):
  q = queries @ W_q            (Q, H)
  k = keys    @ W_k            (KV, H)
  scores[i,j] = sum_h w_v[h] * tanh(q[i,h] + k[j,h])
  attn = softmax_j(scores masked to j < valid_lens[b])
  out  = attn @ values         (Q, V)

Sharding: every core takes query rows [c*64, (c+1)*64) of EVERY batch.
That keeps all 8 cores perfectly balanced and the SPMD program uniform even
though the per-batch key window (truncated to ceil(valid/32)*32 columns --
masked columns contribute exactly 0 after softmax) differs per batch.

Device layout: h on partitions for the tanh stage.  For each query row i,
S[h, j] = k[h, j] + q[h, i] is one DVE tensor_scalar_add (per-partition
scalar broadcast); tanh runs in-place on ScalarE over row-blocks.  The
w_v-weighted reduction over h produces scores TRANSPOSED -- for each
(row, 128-wide j-chunk, h-half) one TensorE matmul with the tanh tile as
stationary and the w_v column as the moving operand writes scores_T[j, i]
into PSUM (partition base 0, always legal).  Softmax then works in the
transposed layout: exp(x + mask) is a single ScalarE activation with the
additive mask as per-partition bias, row sums come from a ones-vector
matmul, and the unnormalized exp_T feeds the final values matmul directly
as lhsT (no attention transpose at all); the 1/sum scale is applied to the
output rows as a per-partition DVE scale.
"""

import sys
import types

import numpy as np

NEG = -1.0e6
NCORES = 8
TRACE = False  # test.py flips this to get a profiled run
LAST_RESULT = None  # BassKernelResults stash for test.py


def _install_axon_profile_hook():
    """antenv.axon_hooks is missing from this image; concourse needs it for
    trace=True under axon. Register the ctypes-based NTFF hook manually."""
    import antenv

    if "antenv.axon_hooks" in sys.modules:
        return
    m = types.ModuleType("antenv.axon_hooks")
    m._hook = None

    def _set(h):
        m._hook = h

    def _get():
        return m._hook

    m.set_axon_ntff_profile_hook = _set
    m.get_axon_ntff_profile_hook = _get
    sys.modules["antenv.axon_hooks"] = m
    antenv.axon_hooks = m
    try:
        from trn_agent_boot.trn_boot import _ntff_profile_via_ctypes

        m.set_axon_ntff_profile_hook(
            _ntff_profile_via_ctypes("/opt/axon/libaxon_pjrt.so")
        )
    except Exception:
        pass


def _patch_tile_drain():
    """The walrus build in this image allows at most ONE sync-wait command
    per instruction; Tile's kernel-tail drain carries every vector-clock
    wait on a single drain. Split them across a chain of drains."""
    import concourse.mybir as mybir
    import concourse.tile as tile
    from concourse.vector_clock import ScopedClock

    if getattr(tile.TileContext, "_drain_patched", False):
        return

    def _drain_and_barrier_chunked(self, tick_clock, wait_clock):
        d0 = self.nc.sync.drain()
        wait_clock.add_sem_waits(d0.ins, ScopedClock({None: tick_clock.global_clock}))
        si = d0.ins.sync_info
        waits = list(si.on_wait) if si is not None else []
        if len(waits) > 1:
            # spread the waits round-robin over all engine streams (each
            # instruction may carry at most one wait for this walrus; a
            # serial SP chain would cost ~27 x wait-resolve latency). The
            # all_engine_barrier right after makes the join equivalent.
            engs = [
                mybir.EngineType.SP,
                mybir.EngineType.DVE,
                mybir.EngineType.Activation,
                mybir.EngineType.PE,
                mybir.EngineType.Pool,
            ]
            d0.ins.sync_info = mybir.SyncInfo(
                on_wait=waits[:1], on_update=list(si.on_update)
            )
            for i in range(1, len(waits)):
                ev = mybir.InstEventSemaphore(
                    name=f"tail-wait-{i}",
                    engine=engs[i % len(engs)],
                    ins=[],
                    outs=[],
                    sync_info=mybir.SyncInfo(on_wait=[waits[i]], on_update=[]),
                )
                self.nc.register_instruction(ev)
                self.nc.cur_bb.bb.add_instruction(ev)

        self.nc.all_engine_barrier()
        assert self.sems is not None
        popped = self.nc._tile_sem_poison_stack.pop()
        assert popped is self._sem_poison
        self.nc.clear_and_free_semaphores(list(self.sems.allocated().values()))
        self.nc.all_engine_barrier()

    tile.TileContext._drain_and_barrier = _drain_and_barrier_chunked
    tile.TileContext._drain_patched = True


def _split_multi_waits(nc):
    """walrus here allows one sync-wait command per instruction; move extra
    waits onto standalone EventSemaphore instructions (same engine, just
    before the original instruction -- semantically identical since waits
    are monotonic sem-ge conditions)."""
    import concourse.mybir as mybir

    n = 0
    for fn in nc.m.functions:
        for blk in fn.blocks:
            out = []
            for inst in blk.instructions:
                si = inst.sync_info
                waits = list(si.on_wait) if si is not None else []
                if len(waits) > 1:
                    for k, w in enumerate(waits[:-1]):
                        ev = mybir.InstEventSemaphore(
                            name=f"{inst.name}-xw{k}",
                            engine=inst.engine,
                            ins=[],
                            outs=[],
                            sync_info=mybir.SyncInfo(on_wait=[w], on_update=[]),
                        )
                        out.append(ev)
                        n += 1
                    inst.sync_info = mybir.SyncInfo(
                        on_wait=[waits[-1]], on_update=list(si.on_update)
                    )
                out.append(inst)
            blk.instructions = out
    return n


def _ceil_to(x, m):
    return -(-int(x) // m) * m


def _row_block(IB, jmax, last=False):
    """Rows per tanh block: keep ACT calls ~2-6K elems/lane (divisor of IB).
    Capped at 16 rows so the DVE->ACT->PE pipeline stays fine-grained."""
    rb = max(1, min(IB, 16, 6144 // jmax))
    return 1 << (rb.bit_length() - 1)


def _build_program(B, Q, D, KV, V, H, jmaxs, IB):
    """One Bass program, shared by all 8 cores (SPMD; data differs per core).

    jmaxs[b]: truncated key-window width for batch b (multiple of 32).
    IB: query rows per (core, batch) = Q // NCORES.
    """
    import contextlib

    import concourse.bass as bass
    import concourse.mybir as mybir
    import concourse.tile as tile

    f32 = mybir.dt.float32
    bf16 = mybir.dt.bfloat16
    AF = mybir.ActivationFunctionType

    JTOT = int(np.sum(jmaxs))
    joff = np.concatenate([[0], np.cumsum(jmaxs)]).astype(int)  # key-col offsets
    # values are packed per batch at 128-row boundaries (slot layout)
    jpads = [_ceil_to(j, 128) for j in jmaxs]
    voff = np.concatenate([[0], np.cumsum(jpads)]).astype(int)
    VTOT = int(voff[-1])
    nchs = [_ceil_to(j, 128) // 128 for j in jmaxs]  # j-chunks per batch
    moff = np.concatenate([[0], np.cumsum(nchs)]).astype(int)  # maskT col offsets
    NCHTOT = int(moff[-1])
    NQROWS = B * IB  # query rows per core
    DC = D // 128  # contraction chunks for the projections
    HC = H // 128  # h-halves

    # processing order: widest batch first, narrowest last -- the epilogues
    # are software-pipelined one batch behind, so the tail is the (short)
    # last batch's epilogue chain.
    order = list(np.argsort(jmaxs))[::-1]

    nc = bass.Bass("TRN2", target_bir_lowering=False)
    d_queriesT = nc.declare_dram_parameter("queriesT", [D, NQROWS], bf16, isOutput=False)
    d_keysT = nc.declare_dram_parameter("keysT", [D, JTOT], bf16, isOutput=False)
    d_values = nc.declare_dram_parameter("values_p", [VTOT, V], bf16, isOutput=False)
    d_wq = nc.declare_dram_parameter("W_q", [D, H], bf16, isOutput=False)
    d_wk = nc.declare_dram_parameter("W_k", [D, H], bf16, isOutput=False)
    d_wv = nc.declare_dram_parameter("wv2", [128, HC], f32, isOutput=False)
    d_maskT = nc.declare_dram_parameter("maskT", [128, NCHTOT], f32, isOutput=False)
    d_out = nc.declare_dram_parameter("out", [NQROWS, V], f32, isOutput=True)

    with tile.TileContext(nc) as tc:
        ctx = contextlib.ExitStack()
        with ctx:
            const_pool = ctx.enter_context(tc.tile_pool(name="const", bufs=1))
            w_pool = ctx.enter_context(tc.tile_pool(name="w", bufs=1))
            in_pool = ctx.enter_context(tc.tile_pool(name="in", bufs=1))
            proj_pool = ctx.enter_context(tc.tile_pool(name="proj", bufs=1))

            wv_sb = const_pool.tile([128, HC], f32)
            nc.gpsimd.dma_start(out=wv_sb[:], in_=d_wv[:])
            wv_bf = const_pool.tile([128, HC], bf16)
            nc.vector.tensor_copy(wv_bf[:], wv_sb[:])
            maskT_sb = const_pool.tile([128, NCHTOT], f32)
            nc.gpsimd.dma_start(out=maskT_sb[:], in_=d_maskT[:])
            ones_sb = const_pool.tile([128, 1], f32)
            nc.gpsimd.memset(ones_sb[:], 1.0)
            ones_bf = const_pool.tile([128, 1], bf16)
            nc.gpsimd.memset(ones_bf[:], 1.0)
            warm = const_pool.tile([1, 1], f32)
            nc.scalar.activation(warm[0:1, 0:1], ones_sb[0:1, 0:1], AF.Tanh)

            kT_all = in_pool.tile([128, DC * JTOT], bf16, tag="kT", name="kT_all")
            wq_all = w_pool.tile([128, DC * H], bf16, tag="wq", name="wq_all")
            wk_all = w_pool.tile([128, DC * H], bf16, tag="wk", name="wk_all")
            qT_all = in_pool.tile([128, DC * NQROWS], bf16, tag="qT", name="qT_all")

            def kt_3d(jo, jm):
                base = kT_all[:]
                return bass.AP(
                    base.tensor, base.offset + jo, [base.ap[0], [JTOT, DC], [1, jm]]
                )

            # one wide DMA per logical tensor: per-tensor 3D access patterns
            # put the dc-chunks side by side in SBUF; the serial ~600ns
            # per-dma_start sequencer issue cost was dominating the head.
            b0p = order[0]
            nc.sync.dma_start(
                out=kt_3d(int(joff[b0p]), int(jmaxs[b0p])),
                in_=d_keysT[:, joff[b0p]:joff[b0p] + jmaxs[b0p]].rearrange(
                    "(dc p) j -> p dc j", p=128
                ),
            )
            nc.scalar.dma_start(
                out=wk_all[:].rearrange("p (dc h) -> p dc h", h=H),
                in_=d_wk.rearrange("(dc p) h -> p dc h", p=128),
            )
            nc.sync.dma_start(
                out=qT_all[:].rearrange("p (dc r) -> p dc r", r=NQROWS),
                in_=d_queriesT.rearrange("(dc p) r -> p dc r", p=128),
            )
            nc.scalar.dma_start(
                out=wq_all[:].rearrange("p (dc h) -> p dc h", h=H),
                in_=d_wq.rearrange("(dc p) h -> p dc h", p=128),
            )
            for b in order[1:]:
                nc.sync.dma_start(
                    out=kt_3d(int(joff[b]), int(jmaxs[b])),
                    in_=d_keysT[:, joff[b]:joff[b] + jmaxs[b]].rearrange(
                        "(dc p) j -> p dc j", p=128
                    ),
                )

            values_sb = in_pool.tile([128, (VTOT // 128) * V], bf16, tag="vals")
            nc.gpsimd.dma_start(
                out=values_sb[:].rearrange("p (s v) -> p s v", v=V),
                in_=d_values.rearrange("(s p) v -> p s v", p=128),
            )

            # ---- projections (per batch window, first-processed first)
            # q/k slabs feed the DVE broadcast-add: k in bf16 (4x DVE mode),
            # q stays f32 (tensor_scalar scalar operand must be f32)
            q_sb = [
                proj_pool.tile([128, NQROWS], f32, tag=f"q{hc}", name=f"qsb{hc}")
                for hc in range(HC)
            ]
            k_sb = [
                proj_pool.tile([128, JTOT], bf16, tag=f"k{hc}", name=f"ksb{hc}")
                for hc in range(HC)
            ]
            if True:
                ppsum = ctx.enter_context(tc.tile_pool(name="ppsum", bufs=1, space="PSUM"))
                def proj_k(b, hcs=None):
                    jo, jm = int(joff[b]), int(jmaxs[b])
                    for hc in hcs if hcs is not None else range(HC):
                        pk = ppsum.tile([128, 512], f32, tag="pproj", name="pk")
                        for dc in range(DC):
                            nc.tensor.matmul(
                                pk[:, :jm],
                                wk_all[:, dc * H + hc * 128:dc * H + (hc + 1) * 128],
                                kT_all[:, dc * JTOT + jo:dc * JTOT + jo + jm],
                                start=(dc == 0),
                                stop=(dc == DC - 1),
                            )
                        nc.vector.tensor_copy(k_sb[hc][:, jo:jo + jm], pk[:, :jm])

                def proj_q(hc):
                    pq = ppsum.tile([128, NQROWS], f32, tag="pproj", name="pq")
                    for dc in range(DC):
                        nc.tensor.matmul(
                            pq[:],
                            wq_all[:, dc * H + hc * 128:dc * H + (hc + 1) * 128],
                            qT_all[:, dc * NQROWS:(dc + 1) * NQROWS],
                            start=(dc == 0),
                            stop=(dc == DC - 1),
                        )
                    nc.vector.tensor_copy(q_sb[hc][:], pq[:])

                proj_k(order[0], hcs=[0])
                proj_q(0)
                proj_k(order[0], hcs=list(range(1, HC)))
                for hc in range(1, HC):
                    proj_q(hc)
                late_projs = [lambda b=b: proj_k(b) for b in order[1:]]

            # ---- main: tanh features -> transposed scores -> softmax -> out
            # S/F slot = biggest row-block; keep total S+F pool usage under
            # ~110KB/partition so worst-case valid_lens still fit SBUF
            slot = max(
                _row_block(IB, int(j), last=(bb == order[-1])) * int(j) * 2
                for bb, j in enumerate(jmaxs)
            )
            s_bufs = max(3, min(8, (110 * 1024) // (2 * slot)))
            s_pool = ctx.enter_context(tc.tile_pool(name="S", bufs=s_bufs))
            sc_psum = ctx.enter_context(tc.tile_pool(name="scps", bufs=5, space="PSUM"))
            sm_psum = ctx.enter_context(tc.tile_pool(name="smps", bufs=1, space="PSUM"))
            o_psum = ctx.enter_context(tc.tile_pool(name="ops", bufs=1, space="PSUM"))
            soft_pool = ctx.enter_context(tc.tile_pool(name="soft", bufs=4))
            out_pool = ctx.enter_context(tc.tile_pool(name="outp", bufs=2))

            def epilogue(b, psc):
                jmax = int(jmaxs[b])
                nch = nchs[b]
                lns = [min(128, jmax - jc * 128) for jc in range(nch)]
                eT = [
                    soft_pool.tile([128, IB], bf16, tag="eT", name=f"eT{b}_{jc}")
                    for jc in range(nch)
                ]
                for jc in range(nch):
                    nc.scalar.activation(
                        eT[jc][: lns[jc], :],
                        psc[jc][: lns[jc], :],
                        AF.Exp,
                        bias=maskT_sb[: lns[jc], moff[b] + jc:moff[b] + jc + 1],
                    )
                psums = sm_psum.tile([1, IB], f32, tag="sm", name=f"psums{b}")
                for jc in range(nch):
                    nc.tensor.matmul(
                        psums[0:1, :],
                        ones_bf[: lns[jc], 0:1],
                        eT[jc][: lns[jc], :],
                        start=(jc == 0),
                        stop=(jc == nch - 1),
                    )
                rs = soft_pool.tile([1, IB], f32, tag="rs", name=f"rs{b}")
                nc.vector.reciprocal(rs[0:1, :], psums[0:1, :])
                prt = sm_psum.tile([IB, 1], f32, tag="sm", name=f"prt{b}")
                nc.tensor.matmul(
                    prt[:, 0:1], rs[0:1, :], ones_sb[0:1, 0:1], start=True, stop=True
                )
                rinv = soft_pool.tile([IB, 1], f32, tag="rinv", name=f"rinv{b}")
                nc.vector.tensor_copy(rinv[:], prt[:])

                pout = o_psum.tile([IB, V], f32, tag="pout", name=f"pout{b}")
                for jc in range(nch):
                    nc.tensor.matmul(
                        pout[:],
                        eT[jc][: lns[jc], :],
                        values_sb[: lns[jc], (voff[b] // 128 + jc) * V:(voff[b] // 128 + jc + 1) * V],
                        start=(jc == 0),
                        stop=(jc == nch - 1),
                    )
                out_sb = out_pool.tile([IB, V], f32, tag="osb", name=f"osb{b}")
                nc.vector.tensor_scalar_mul(out_sb[:], pout[:], rinv[:])
                nc.sync.dma_start(out=d_out[b * IB:(b + 1) * IB, :], in_=out_sb[:])

            pending = None  # (b, psc) whose epilogue is deferred one batch
            for b in order:
                jmax = int(jmaxs[b])
                jo = int(joff[b])
                nch = nchs[b]
                lns = [min(128, jmax - jc * 128) for jc in range(nch)]
                RB = _row_block(IB, jmax, last=(b == order[-1]))

                psc = [
                    sc_psum.tile([128, IB], f32, tag="pscT", name=f"pscT{b}_{jc}")
                    for jc in range(nch)
                ]
                blocks = []
                r = 0
                while r < IB:
                    if r + RB >= IB and RB > 8:
                        blocks += [(r, RB // 2), (r + RB // 2, RB - RB // 2)]
                        r += RB
                    else:
                        blocks.append((r, RB))
                        r += RB
                for r0, rb in blocks:
                    S = [
                        s_pool.tile(
                            [128, rb * jmax], bf16, tag="S", name=f"S{b}_{r0}_{hcx}"
                        )
                        for hcx in range(HC)
                    ]
                    F = [
                        s_pool.tile(
                            [128, rb * jmax], bf16, tag="F", name=f"F{b}_{r0}_{hcx}"
                        )
                        for hcx in range(HC)
                    ]
                    for hc in range(HC):
                        if jmax <= 128:
                            # narrow window: one broadcast tensor-tensor add
                            # covers the whole row block (per-call DVE
                            # overhead would dominate row-by-row adds)
                            kb = k_sb[hc][:, jo:jo + jmax]
                            k_rep = bass.AP(
                                kb.tensor, kb.offset, [kb.ap[0], [0, rb], kb.ap[1]]
                            )
                            qb = q_sb[hc][:, b * IB + r0:b * IB + r0 + rb]
                            q_rep = bass.AP(
                                qb.tensor, qb.offset, [qb.ap[0], qb.ap[1], [0, jmax]]
                            )
                            sb = S[hc][:, : rb * jmax]
                            s3 = bass.AP(
                                sb.tensor, sb.offset, [sb.ap[0], [jmax, rb], [1, jmax]]
                            )
                            nc.vector.tensor_add(s3, k_rep, q_rep)
                        else:
                            for m in range(rb):
                                row = b * IB + r0 + m
                                nc.vector.tensor_scalar_add(
                                    S[hc][:, m * jmax:(m + 1) * jmax],
                                    k_sb[hc][:, jo:jo + jmax],
                                    q_sb[hc][:, row:row + 1],
                                )
                        # bf16 tanh output: full-128-col weights trigger the
                        # compiler-automatic FWL fast-weight-load path
                        nc.scalar.activation(F[hc][:], S[hc][:], AF.Tanh)
                    for m in range(rb):
                        for jc in range(nch):
                            for hc in range(HC):
                                nc.tensor.matmul(
                                    psc[jc][: lns[jc], r0 + m:r0 + m + 1],
                                    F[hc][:, m * jmax + jc * 128:m * jmax + jc * 128 + lns[jc]],
                                    wv_bf[:, hc:hc + 1],
                                    start=(hc == 0),
                                    stop=(hc == HC - 1),
                                )
                    if r0 == 0:
                        if pending is not None:
                            epilogue(*pending)
                            pending = None
                        while late_projs:
                            late_projs.pop(0)()
                pending = (b, psc)
            epilogue(*pending)

    _split_multi_waits(nc)
    return nc


def kernel(queries, keys, values, valid_lens, W_q, W_k, w_v):
    global LAST_RESULT
    _install_axon_profile_hook()
    _patch_tile_drain()
    from concourse.bass_utils import run_bass_kernel_spmd

    import ml_dtypes

    bf = ml_dtypes.bfloat16
    queries = np.ascontiguousarray(queries, dtype=np.float32)
    keys = np.ascontiguousarray(keys, dtype=np.float32)
    values = np.ascontiguousarray(values, dtype=np.float32)
    W_q = np.ascontiguousarray(W_q, dtype=np.float32)
    W_k = np.ascontiguousarray(W_k, dtype=np.float32)
    w_v = np.ascontiguousarray(w_v, dtype=np.float32)
    vl = np.asarray(valid_lens).astype(np.int64)

    B, Q, D = queries.shape
    KV = keys.shape[1]
    V = values.shape[2]
    H = W_q.shape[1]
    IB = Q // NCORES
    HC = H // 128

    jmaxs = [min(KV, _ceil_to(max(int(v), 1), 32)) for v in vl]
    jpads = [_ceil_to(j, 128) for j in jmaxs]
    nchs = [j // 128 for j in jpads]
    VTOT = int(np.sum(jpads))

    nc = _build_program(B, Q, D, KV, V, H, jmaxs, IB)

    # ---- shared (core-independent) arrays
    keysT = np.concatenate(
        [keys[b, : jmaxs[b], :].T for b in range(B)], axis=1
    ).astype(bf)  # (D, JTOT)
    values_p = np.zeros((VTOT, V), bf)
    off = 0
    for b in range(B):
        values_p[off:off + jmaxs[b]] = values[b, : jmaxs[b], :].astype(bf)
        off += jpads[b]
    wv2 = w_v.reshape(HC, 128).T.copy()  # (128, HC)
    # additive mask in the transposed layout: one 128-long column per
    # (batch, j-chunk); row p of column (b, jc) corresponds to key j = jc*128+p
    mcols = []
    for b in range(B):
        for jc in range(nchs[b]):
            j = jc * 128 + np.arange(128)
            mcols.append(np.where(j < int(vl[b]), 0.0, NEG).astype(np.float32))
    maskT = np.stack(mcols, axis=1)  # (128, NCHTOT)

    in_maps = []
    for c in range(NCORES):
        queriesT = np.concatenate(
            [queries[b, c * IB:(c + 1) * IB, :].T for b in range(B)], axis=1
        )  # (D, B*IB)
        in_maps.append(
            {
                "queriesT": np.ascontiguousarray(queriesT.astype(bf)),
                "keysT": np.ascontiguousarray(keysT),
                "values_p": values_p,
                "W_q": W_q.astype(bf),
                "W_k": W_k.astype(bf),
                "wv2": wv2,
                "maskT": maskT,
            }
        )

    res = run_bass_kernel_spmd(
        nc, in_maps, core_ids=list(range(NCORES)), trace=TRACE
    )
    LAST_RESULT = res

    out = np.empty((B, Q, V), np.float32)
    for c in range(NCORES):
        o = res.results[c]["out"]  # (B*IB, V)
        for b in range(B):
            out[b, c * IB:(c + 1) * IB, :] = o[b * IB:(b + 1) * IB, :]
    return out
